# revision 23
# baseline (speedup 1.0000x reference)
"""Multi-head latent attention kernel for Trainium2, 8 NeuronCores.

Problem (hardcoded shapes):
  hidden_states [2, 2048, 4096] f32, attention_mask [1,1,2048,2048] f32,
  Wq [4096,4096], Wk/Wv [4096,1024], Wo [4096,4096].
  4 query heads x 1024 head_dim, 1 kv head, interleaved RoPE, softmax, o-proj.

Sharding: core c = (batch b=c//4, quarter r=c%4), all within-batch groups of 4.
  - k^T / v computed from the core's sequence quarter (hsq input) and
    AllGathered (one combined collective) within the 4-core batch group.
  - Attention is sharded over QUERY positions: core r handles global 256-row
    i-blocks (r, 7-r) for ALL 4 heads, so the output projection is fully
    local. slot0 = block r (rows < 1024), slot1 = block 7-r (rows >= 1024).

Mixed precision: slot0 (early rows, softmax-peaked -> noise-sensitive) runs
entirely in bf16. slot1 (late rows, flat softmax averages out fp8 noise)
uses fp8e4m3 DoubleRow matmuls (2x PE throughput) for the q projection,
probs@V, and the output projection; q@k scores stay bf16 (score noise on
competitive rows is the dominant error channel - validated by numpy sim,
rel err 6.6e-3 vs gate 2e-2). fp8 DoubleRow fuses 2 contraction planes per
instruction: stationary [128,2,M], moving [128,2,N] -> out [M,N]. The host
pre-interleaves wq8/wo8/hs8 into pair-plane layout. Probs are computed as
exp(SCALE*s - 2) to stay under fp8 max 240; the shift cancels in the
softmax normalization (denominator computed from the quantized probs).
"""

import numpy as np
import ml_dtypes

from concourse import bass, mybir, tile, bacc
from concourse import bass_utils

BF16 = mybir.dt.bfloat16
F8 = mybir.dt.float8e4
F32 = mybir.dt.float32
NP_F8 = ml_dtypes.float8_e4m3

B, S, H = 2, 2048, 4096
NH, D = 4, 1024  # query heads, head dim
PD = D // 2  # rope pair count (512)
SCALE = D ** -0.5
PSHIFT = 2.0  # exp bias for fp8 probs (keeps max under fp8e4m3 240)
NCORES = 8
GROUPS = [[0, 1, 2, 3], [4, 5, 6, 7]]

KC = H // 128  # 32 contraction chunks over hidden
DC = D // 128  # 8 d-chunks of head dim
JC = S // 128  # 16 j-chunks (key) of 128
Q = S // 4  # 512, per-core kv sequence quarter

DR = mybir.MatmulPerfMode.DoubleRow

# results of the traced+profiled run (filled by kernel() when trace=True)
LAST_RESULTS = None


def _build(cats, n_mixed):
    """Build the SPMD bass program.

    cats: dict (lb, jc) -> "skip" | "clean" | int (index into packed mask
    tiles); lb in {0,1} is the local 256-row i-block, union over cores.
    lb=0 is the early block (bf16), lb=1 the late block (fp8 path).
    """
    nc = bacc.Bacc("TRN2", target_bir_lowering=False, debug=False,
                   num_devices=NCORES)

    hsq_d = nc.dram_tensor("hsq", [128, KC * Q], BF16, kind="ExternalInput")
    hsq2_d = nc.dram_tensor("hsq2", [128, KC * 256], BF16, kind="ExternalInput")
    hs8_d = nc.dram_tensor("hs8", [128, 16 * 512], F8, kind="ExternalInput")
    wq_d = nc.dram_tensor("wq", [128, KC * H], BF16, kind="ExternalInput")
    wk_d = nc.dram_tensor("wk", [128, KC * D], BF16, kind="ExternalInput")
    wv_d = nc.dram_tensor("wv", [128, KC * D], BF16, kind="ExternalInput")
    wo_d = nc.dram_tensor("wo", [128, KC * H], BF16, kind="ExternalInput")
    wo8_d = nc.dram_tensor("wo8", [2048, 2 * H], F8, kind="ExternalInput")
    cosq_d = nc.dram_tensor("cosq", [PD, Q], BF16, kind="ExternalInput")
    sinq_d = nc.dram_tensor("sinq", [PD, Q], BF16, kind="ExternalInput")
    cosq2_d = nc.dram_tensor("cosq2", [PD, 512], BF16, kind="ExternalInput")
    sinq2_d = nc.dram_tensor("sinq2", [PD, 512], BF16, kind="ExternalInput")
    nmask = max(n_mixed, 1)
    maskp_d = nc.dram_tensor("maskp", [nmask * 128, 256], F32,
                             kind="ExternalInput")
    out_d = nc.dram_tensor("out", [512, H], F32, kind="ExternalOutput")

    # combined k+v collective bounce: rows [0:1024] = kT slice [1024, Q],
    # rows [1024:2048] = v slice [Q, 1024] flattened row-major to [1024, 512]
    # (a single collective: split k/v AllGathers land on a DMA ring that
    # blocks all later input DMAs -- measured 90us+ of pure queue stall)
    kv_in = nc.dram_tensor("kv_in", [2048, 512], BF16, kind="Internal")
    kv_out = nc.dram_tensor("kv_out", [4 * 2048, 512], BF16, kind="Internal")

    PAIRS = [(0, 4), (1, 5), (2, 6), (3, 7)]

    with tile.TileContext(nc) as tc:
        with tc.tile_pool(name="pers", bufs=1) as pers:
            ones_col = pers.tile([128, 1], BF16, name="ones_col",
                                 tag="ones_col")
            nc.vector.memset(ones_col[:], 1.0)
            # DR ldweights needs outer free step 16B-aligned -> pad to 16
            ones8 = pers.tile([128, 2, 16], F8, name="ones8", tag="ones8")
            nc.vector.memset(ones8[:], 1.0)
            ones_row = pers.tile([1, 128], F32, name="ones_row",
                                 tag="ones_row")
            nc.vector.memset(ones_row[:], 1.0)
            ebias = pers.tile([128, 1], F32, name="ebias", tag="ebias")
            nc.vector.memset(ebias[:], -PSHIFT)
            pq = pers
            # q^T per slot: 32 chunks [128 d, 256 i] (4 heads x 8 dc)
            qT0 = [pq.tile([128, 256], BF16, name=f"qt0_{i}",
                           tag=f"qt0_{i}") for i in range(4 * DC)]
            qT1 = [pq.tile([128, 256], BF16, name=f"qt1_{i}",
                           tag=f"qt1_{i}") for i in range(4 * DC)]
            kT = [pq.tile([128, S], BF16, name=f"kt{i}", tag=f"kt{i}")
                  for i in range(DC)]
            # v for slot0 (bf16, first 8 jc only) and slot1 (fp8 jc pairs)
            vT0 = [pq.tile([128, D], BF16, name=f"vt0_{i}", tag=f"vt0_{i}")
                   for i in range(JC // 2)]
            v8 = [pq.tile([128, 2, D], F8, name=f"v8_{i}", tag=f"v8_{i}")
                  for i in range(JC // 2)]
            # mask tiles resident in SBUF, loaded once, shared across heads
            nmt = max(n_mixed, 1)
            mask_t = [pq.tile([128, 256], F32, name=f"mk{i}", tag=f"mk{i}")
                      for i in range(nmt)]

            # ============ phase A1: k/v projections + kv AG ============
            with (
                tc.tile_pool(name="pa1", bufs=3) as pa1,
                tc.tile_pool(name="paps", bufs=8, space="PSUM") as paps,
            ):
                hqc = [pa1.tile([128, 8, Q], BF16, name=f"hqc{i}",
                                tag=f"hqc{i}", bufs=1) for i in range(4)]
                for i in range(4):
                    nc.sync.dma_start(
                        hqc[i][:],
                        hsq_d[:, 4096 * i:4096 * (i + 1)].rearrange(
                            "p (kc s) -> p kc s", kc=8))

                def rope_pair(pool, ps_e, ps_o, c_t, s_t, out_e, out_o, n):
                    """Stage psum pair to bf16, apply rope, write outputs."""
                    st_e = pool.tile([128, n], BF16, name="stg", tag="stg",
                                     bufs=6)
                    st_o = pool.tile([128, n], BF16, name="stg", tag="stg",
                                     bufs=6)
                    nc.scalar.activation(st_e[:], ps_e[:],
                                         mybir.ActivationFunctionType.Copy)
                    nc.scalar.activation(st_o[:], ps_o[:],
                                         mybir.ActivationFunctionType.Copy)
                    t1 = pool.tile([128, n], BF16, name="rtmp", tag="rtmp",
                                   bufs=4)
                    t2 = pool.tile([128, n], BF16, name="rtmp", tag="rtmp",
                                   bufs=4)
                    nc.vector.tensor_mul(t1[:], st_e[:], c_t)
                    nc.vector.tensor_mul(t2[:], st_o[:], s_t)
                    nc.vector.tensor_sub(out_e, t1[:], t2[:])
                    t3 = pool.tile([128, n], BF16, name="rtmp", tag="rtmp",
                                   bufs=4)
                    t4 = pool.tile([128, n], BF16, name="rtmp", tag="rtmp",
                                   bufs=4)
                    nc.vector.tensor_mul(t3[:], st_o[:], c_t)
                    nc.vector.tensor_mul(t4[:], st_e[:], s_t)
                    nc.vector.tensor_add(out_o, t3[:], t4[:])

                # --- K pass: kc-outer over 8 psum banks, then rope pairs ---
                kps = [paps.tile([128, Q], F32, name="mmps", tag="mmps")
                       for _ in range(DC)]
                for kcg in range(KC // 4):
                    wkt = pa1.tile([128, 4, D], BF16, name="wk", tag="wk",
                                   bufs=2)
                    nc.sync.dma_start(
                        wkt[:], wk_d[:, 4096 * kcg:4096 * (kcg + 1)].rearrange(
                            "p (ki c) -> p ki c", ki=4))
                    for ki in range(4):
                        kc = 4 * kcg + ki
                        for dc in range(DC):
                            nc.tensor.matmul(
                                kps[dc][:],
                                wkt[:, ki, 128 * dc:128 * (dc + 1)],
                                hqc[kc // 8][:, kc % 8, :],
                                start=(kc == 0), stop=(kc == KC - 1))
                for pi, (de, do) in enumerate(PAIRS):
                    c_t = pa1.tile([128, Q], BF16, name="ckt", tag="ckt",
                                   bufs=2)
                    s_t = pa1.tile([128, Q], BF16, name="skt", tag="skt",
                                   bufs=2)
                    nc.sync.dma_start(c_t[:],
                                      cosq_d[128 * pi:128 * (pi + 1), :])
                    nc.sync.dma_start(s_t[:],
                                      sinq_d[128 * pi:128 * (pi + 1), :])
                    ke = pa1.tile([128, Q], BF16, name="kout", tag="kout",
                                  bufs=4)
                    ko = pa1.tile([128, Q], BF16, name="kout", tag="kout",
                                  bufs=4)
                    rope_pair(pa1, kps[de], kps[do], c_t[:], s_t[:],
                              ke[:], ko[:], Q)
                    nc.sync.dma_start(kv_in[128 * de:128 * (de + 1), :],
                                      ke[:])
                    nc.sync.dma_start(kv_in[128 * do:128 * (do + 1), :],
                                      ko[:])

                # --- V pass: kc-outer over 8 psum banks ---
                vps = [paps.tile([128, 512], F32, name="mmps", tag="mmps")
                       for _ in range(8)]
                for kcg in range(KC // 4):
                    wvt = pa1.tile([128, 4, D], BF16, name="wv", tag="wv",
                                   bufs=2)
                    nc.sync.dma_start(
                        wvt[:], wv_d[:, 4096 * kcg:4096 * (kcg + 1)].rearrange(
                            "p (ki c) -> p ki c", ki=4))
                    for ki in range(4):
                        kc = 4 * kcg + ki
                        for sc in range(4):
                            for dvb in range(2):
                                nc.tensor.matmul(
                                    vps[sc * 2 + dvb][:],
                                    hqc[kc // 8][:, kc % 8,
                                                 128 * sc:128 * (sc + 1)],
                                    wvt[:, ki, 512 * dvb:512 * (dvb + 1)],
                                    start=(kc == 0), stop=(kc == KC - 1))
                for sc in range(4):
                    for dvb in range(2):
                        vt = pa1.tile([128, 512], BF16, name="vout",
                                      tag="vout", bufs=4)
                        nc.scalar.activation(vt[:], vps[sc * 2 + dvb][:],
                                             mybir.ActivationFunctionType.Copy)
                        dst = kv_in[1024 + 256 * sc:1024 + 256 * (sc + 1), :]
                        dst = dst.rearrange("(p c) f -> p c f", c=2)[:, dvb, :]
                        nc.sync.dma_start(dst, vt[:])

                # --- combined kv AllGather (overlaps q pass) ---
                nc.gpsimd.collective_compute(
                    "AllGather", mybir.AluOpType.bypass, replica_groups=GROUPS,
                    ins=[kv_in.ap().opt()], outs=[kv_out.ap().opt()])
                for i in range(nmt):
                    nc.sync.dma_start(mask_t[i][:],
                                      maskp_d[128 * i:128 * (i + 1), :])

            # ============ phase A2: q projections (slot0 bf16, slot1 fp8) ==
            with (
                tc.tile_pool(name="pa2", bufs=3) as pa2,
                tc.tile_pool(name="paps2", bufs=8, space="PSUM") as paps2,
            ):
                hqc2 = [pa2.tile([128, 8, 256], BF16, name=f"hqc2{i}",
                                 tag=f"hqc2{i}", bufs=1) for i in range(4)]
                for i in range(4):
                    nc.sync.dma_start(
                        hqc2[i][:],
                        hsq2_d[:, 2048 * i:2048 * (i + 1)].rearrange(
                            "p (kc s) -> p kc s", kc=8))
                hs8all = pa2.tile([128, 16, 2, 256], F8, name="hs8all",
                                  tag="hs8all", bufs=1)
                nc.sync.dma_start(
                    hs8all[:],
                    hs8_d.ap().rearrange("p (t i c) -> p t i c", t=16, i=2))
                hs8t = [hs8all[:, t, :, :] for t in range(16)]

                for hp in range(4):
                    # --- slot0 (bf16): 8 dc chunks in 4 psum half-banks ---
                    qps = [paps2.tile([128, 256], F32, name="mmps2",
                                      tag="mmps2") for _ in range(DC)]
                    wq8cs = []
                    for kcg in range(KC // 4):
                        wqt = pa2.tile([128, 4, D], BF16, name="wqs",
                                       tag="wqs", bufs=3)
                        nc.sync.dma_start(
                            wqt[:],
                            wq_d[:, 16384 * kcg + 4096 * hp:
                                 16384 * kcg + 4096 * (hp + 1)].rearrange(
                                "p (ki c) -> p ki c", ki=4))
                        wq8c = pa2.tile([128, 2, 2, D], F8, name="wq8c",
                                        tag="wq8c", bufs=8)
                        for tp in range(2):
                            for pl in range(2):
                                nc.vector.tensor_copy(
                                    wq8c[:, tp, pl, :],
                                    wqt[:, 2 * tp + pl, :])
                        wq8cs.append(wq8c)
                        for ki in range(4):
                            kc = 4 * kcg + ki
                            for dc in range(DC):
                                nc.tensor.matmul(
                                    qps[dc][:],
                                    wqt[:, ki, 128 * dc:128 * (dc + 1)],
                                    hqc2[kc // 8][:, kc % 8, :],
                                    start=(kc == 0), stop=(kc == KC - 1))
                    for pi, (de, do) in enumerate(PAIRS):
                        c_t = pa2.tile([128, 256], BF16, name="cq0",
                                       tag="cq0", bufs=3)
                        s_t = pa2.tile([128, 256], BF16, name="sq0",
                                       tag="sq0", bufs=3)
                        nc.sync.dma_start(
                            c_t[:], cosq2_d[128 * pi:128 * (pi + 1), 0:256])
                        nc.sync.dma_start(
                            s_t[:], sinq2_d[128 * pi:128 * (pi + 1), 0:256])
                        rope_pair(pa2, qps[de], qps[do], c_t[:], s_t[:],
                                  qT0[DC * hp + de][:], qT0[DC * hp + do][:],
                                  256)
                    # --- slot1 (fp8 DoubleRow): weights converted on-chip
                    # from the bf16 wqt tiles by the (otherwise idle) DVE ---
                    qps8 = [paps2.tile([128, 256], F32, name="mmps2",
                                       tag="mmps2") for _ in range(DC)]
                    for t in range(16):
                        kcg, tp = t // 2, t % 2
                        for dc in range(DC):
                            nc.tensor.matmul(
                                qps8[dc][:],
                                wq8cs[kcg][:, tp, :, 128 * dc:128 * (dc + 1)],
                                hs8t[t],
                                start=(t == 0), stop=(t == 15),
                                perf_mode=DR)
                    for pi, (de, do) in enumerate(PAIRS):
                        c_t = pa2.tile([128, 256], BF16, name="cq1",
                                       tag="cq1", bufs=3)
                        s_t = pa2.tile([128, 256], BF16, name="sq1",
                                       tag="sq1", bufs=3)
                        nc.sync.dma_start(
                            c_t[:], cosq2_d[128 * pi:128 * (pi + 1), 256:512])
                        nc.sync.dma_start(
                            s_t[:], sinq2_d[128 * pi:128 * (pi + 1), 256:512])
                        rope_pair(pa2, qps8[de], qps8[do], c_t[:], s_t[:],
                                  qT1[DC * hp + de][:], qT1[DC * hp + do][:],
                                  256)
                    if hp == 2:
                        # kT quarters 0-1: queue reaches here ~3/4 through
                        # the q-pass DMA stream, after the AllGather ends
                        for r in (0, 1):
                            for dc in range(DC):
                                nc.sync.dma_start(
                                    kT[dc][:, Q * r:Q * (r + 1)],
                                    kv_out[2048 * r + 128 * dc:
                                           2048 * r + 128 * (dc + 1), :])
                    if hp == 3:
                        for r in (2, 3):
                            for dc in range(DC):
                                nc.sync.dma_start(
                                    kT[dc][:, Q * r:Q * (r + 1)],
                                    kv_out[2048 * r + 128 * dc:
                                           2048 * r + 128 * (dc + 1), :])

            # --- remaining kT/v loads in attention need-order ---
            with tc.tile_pool(name="vload", bufs=2) as vload:
                def load_v(jc):
                    base = 2048 * (jc // 4) + 1024 + 256 * (jc % 4)
                    vsrc = kv_out[base:base + 256, :].rearrange(
                        "(p c) f -> p (c f)", c=2)
                    if jc < 8:
                        nc.sync.dma_start(vT0[jc][:], vsrc)
                        nc.vector.tensor_copy(v8[jc // 2][:, jc % 2, :],
                                              vT0[jc][:])
                    else:
                        vtmp = vload.tile([128, D], BF16, name="vtmp",
                                          tag="vtmp", bufs=2)
                        nc.sync.dma_start(vtmp[:], vsrc)
                        nc.vector.tensor_copy(v8[jc // 2][:, jc % 2, :],
                                              vtmp[:])

                for jc in range(JC):
                    load_v(jc)

            # ========== phase B+C: attention then o-proj ==========
            # Order: slot0 attn, slot1 attn (DMA-light, lets the wo stream
            # prefetch), then both o-proj passes (DMA-heavy, run exclusive).
            with tc.tile_pool(name="pb", bufs=2) as pb:
                attnT0 = [pb.tile([128, 256], BF16, name=f"at0_{i}",
                                  tag=f"at0_{i}", bufs=1) for i in range(KC)]
                attnT8 = [pb.tile([128, 2, 256], F8, name=f"at8_{i}",
                                  tag=f"at8_{i}", bufs=1)
                          for i in range(KC // 2)]

                live0 = [jc for jc in range(JC) if cats[(0, jc)] != "skip"]
                with tc.tile_pool(name="psa", bufs=2, space="PSUM") as psa:
                    for slot in range(2):
                        for h in range(NH):
                            # ---- scores + exp ----
                            if slot == 0:
                                pT = {}
                                for jc in live0:
                                    sps = psa.tile([128, 256], F32,
                                                   name="sps", tag="sps",
                                                   bufs=3)
                                    for dc in range(DC):
                                        nc.tensor.matmul(
                                            sps[:],
                                            kT[dc][:,
                                                   128 * jc:128 * (jc + 1)],
                                            qT0[DC * h + dc][:],
                                            start=(dc == 0),
                                            stop=(dc == DC - 1))
                                    cat = cats[(0, jc)]
                                    if isinstance(cat, int):
                                        nc.vector.tensor_add(
                                            sps[:], sps[:], mask_t[cat][:])
                                    pt = pb.tile([128, 256], BF16,
                                                 name=f"pt{jc}",
                                                 tag=f"pt{jc}", bufs=2)
                                    nc.scalar.activation(
                                        pt[:], sps[:],
                                        mybir.ActivationFunctionType.Exp,
                                        scale=SCALE)
                                    pT[jc] = pt
                            else:
                                p8 = {}
                                for pr in range(JC // 2):
                                    p8[pr] = pb.tile(
                                        [128, 2, 256], F8, name=f"p8_{pr}",
                                        tag=f"p8_{pr}", bufs=2)
                                for jc in range(JC):
                                    cat = cats[(1, jc)]
                                    if cat == "skip":
                                        nc.vector.memset(
                                            p8[jc // 2][:, jc % 2, :], 0.0)
                                        continue
                                    sps = psa.tile([128, 256], F32,
                                                   name="sps", tag="sps",
                                                   bufs=3)
                                    for dc in range(DC):
                                        nc.tensor.matmul(
                                            sps[:],
                                            kT[dc][:,
                                                   128 * jc:128 * (jc + 1)],
                                            qT1[DC * h + dc][:],
                                            start=(dc == 0),
                                            stop=(dc == DC - 1))
                                    if isinstance(cat, int):
                                        nc.vector.tensor_add(
                                            sps[:], sps[:], mask_t[cat][:])
                                    nc.scalar.activation(
                                        p8[jc // 2][:, jc % 2, :], sps[:],
                                        mybir.ActivationFunctionType.Exp,
                                        scale=SCALE, bias=ebias[:])

                            # ---- PV with interleaved denominator ----
                            r_sb = pb.tile([1, 256], F32, name="rsb",
                                           tag="rsb", bufs=2)
                            rbc = pb.tile([128, 256], F32, name="rbc",
                                          tag="rbc", bufs=2)
                            held = []  # (dc2, pvps) awaiting rbc

                            def emit_pv(dc2):
                                pvps = psa.tile([128, 256], F32, name="pvps",
                                                tag="pvps", bufs=3)
                                if slot == 0:
                                    for n, jc in enumerate(live0):
                                        nc.tensor.matmul(
                                            pvps[:],
                                            vT0[jc][:, 128 * dc2:
                                                    128 * (dc2 + 1)],
                                            pT[jc][:], start=(n == 0),
                                            stop=(n == len(live0) - 1))
                                else:
                                    for pr in range(JC // 2):
                                        nc.tensor.matmul(
                                            pvps[:],
                                            v8[pr][:, :, 128 * dc2:
                                                   128 * (dc2 + 1)],
                                            p8[pr][:], start=(pr == 0),
                                            stop=(pr == JC // 2 - 1),
                                            perf_mode=DR)
                                return pvps

                            def emit_evac(dc2, pvps):
                                c = DC * h + dc2
                                if slot == 0:
                                    nc.vector.tensor_mul(
                                        attnT0[c][:], pvps[:], rbc[:])
                                else:
                                    nc.vector.tensor_mul(
                                        attnT8[c // 2][:, c % 2, :],
                                        pvps[:], rbc[:])

                            for dc2 in range(DC):
                                pvps = emit_pv(dc2)
                                if dc2 == 0:
                                    # denominator after PV0 (exps all done)
                                    l_ps = psa.tile([16, 256], F32,
                                                    name="lps", tag="lps",
                                                    bufs=1)
                                    if slot == 0:
                                        for n, jc in enumerate(live0):
                                            nc.tensor.matmul(
                                                l_ps[0:1, :], ones_col[:],
                                                pT[jc][:], start=(n == 0),
                                                stop=(n == len(live0) - 1))
                                    else:
                                        for pr in range(JC // 2):
                                            nc.tensor.matmul(
                                                l_ps[:], ones8[:],
                                                p8[pr][:], start=(pr == 0),
                                                stop=(pr == JC // 2 - 1),
                                                perf_mode=DR)
                                    nc.vector.reciprocal(r_sb[:],
                                                         l_ps[0:1, :])
                                    held.append((dc2, pvps))
                                elif dc2 == 1:
                                    # r broadcast: reciprocal done during PV1
                                    r_ps = psa.tile([128, 256], F32,
                                                    name="rps", tag="rps",
                                                    bufs=1)
                                    nc.tensor.matmul(r_ps[:], ones_row[:],
                                                     r_sb[:], start=True,
                                                     stop=True)
                                    nc.scalar.activation(
                                        rbc[:], r_ps[:],
                                        mybir.ActivationFunctionType.Copy)
                                    held.append((dc2, pvps))
                                    for d, p in held:
                                        emit_evac(d, p)
                                    held = []
                                else:
                                    emit_evac(dc2, pvps)

                # ---- o-proj slot0 (bf16) then slot1 (fp8 DR) ----
                # eb-groups of 4 share one [128, 2048] weight tile so each
                # DMA moves 2-4KB per partition row (descriptor-rate bound)
                with (
                    tc.tile_pool(name="pc", bufs=2) as pc,
                    tc.tile_pool(name="psc", bufs=2, space="PSUM") as psc,
                ):
                    for g in range(2):
                        ops = [psc.tile([128, 512], F32, name="ops",
                                        tag="ops", bufs=8)
                               for _ in range(8)]
                        for t in range(KC // 2):
                            wot = pc.tile([128, 2, 2048], BF16, name="wot",
                                          tag="wot", bufs=3)
                            nc.sync.dma_start(
                                wot[:],
                                wo_d[:, 65536 * g + 4096 * t:
                                     65536 * g + 4096 * (t + 1)].rearrange(
                                    "p (ki c) -> p ki c", ki=2))
                            for ki in range(2):
                                kc = 2 * t + ki
                                for e in range(4):
                                    for ic in range(2):
                                        nc.tensor.matmul(
                                            ops[2 * e + ic][:],
                                            attnT0[kc][:, 128 * ic:
                                                       128 * (ic + 1)],
                                            wot[:, ki, 512 * e:512 * (e + 1)],
                                            start=(kc == 0),
                                            stop=(kc == KC - 1))
                        for ic in range(2):
                            ot = pc.tile([128, 2048], F32, name="ot0",
                                         tag="ot", bufs=2)
                            for e in range(4):
                                nc.vector.tensor_copy(
                                    ot[:, 512 * e:512 * (e + 1)],
                                    ops[2 * e + ic][:])
                            nc.sync.dma_start(
                                out_d[128 * ic:128 * (ic + 1),
                                      2048 * g:2048 * (g + 1)], ot[:])
                    for g in range(2):
                        ops = [psc.tile([128, 512], F32, name="ops",
                                        tag="ops", bufs=8)
                               for _ in range(8)]
                        for t in range(KC // 2):
                            w8g = pc.tile([128, 2, 2048], F8, name="w8g",
                                          tag="w8g", bufs=2)
                            nc.sync.dma_start(
                                w8g[:],
                                wo8_d[128 * t:128 * (t + 1),
                                      4096 * g:4096 * (g + 1)].rearrange(
                                    "p (i c) -> p i c", i=2))
                            for e in range(4):
                                for ic in range(2):
                                    nc.tensor.matmul(
                                        ops[2 * e + ic][:],
                                        attnT8[t][:, :,
                                                  128 * ic:128 * (ic + 1)],
                                        w8g[:, :, 512 * e:512 * (e + 1)],
                                        start=(t == 0),
                                        stop=(t == KC // 2 - 1),
                                        perf_mode=DR)
                        for ic in range(2):
                            ot = pc.tile([128, 2048], F32, name="ot1",
                                         tag="ot", bufs=2)
                            for e in range(4):
                                nc.vector.tensor_copy(
                                    ot[:, 512 * e:512 * (e + 1)],
                                    ops[2 * e + ic][:])
                            nc.sync.dma_start(
                                out_d[256 + 128 * ic:256 + 128 * (ic + 1),
                                      2048 * g:2048 * (g + 1)], ot[:])

    nc.compile()
    return nc


_BUILD_CACHE = {}

# core r (within its batch group) handles global 256-row i-blocks (r, 7-r)
GMAP = [(r, 7 - r) for r in range(4)]


def _classify_mask(mask):
    """Union-classify each (local block lb, jc) over the 4 quarter cores.

    Returns (cats, per-core packed mask tile arrays, n_mixed). The program
    structure (cats) is shared by all cores; mask tiles are per-core data.
    """
    m = np.asarray(mask).reshape(S, S)  # [i, j]
    cats = {}
    tiles = [[] for _ in range(4)]
    n = 0
    for lb in range(2):
        for jc in range(JC):
            blks = [m[256 * GMAP[r][lb]:256 * (GMAP[r][lb] + 1),
                      128 * jc:128 * (jc + 1)] for r in range(4)]
            if all(np.all(b <= -1e8) for b in blks):
                cats[(lb, jc)] = "skip"
            elif not any(b.any() for b in blks):
                cats[(lb, jc)] = "clean"
            else:
                cats[(lb, jc)] = n
                n += 1
                for r in range(4):
                    # [j, i] orientation, prescaled by 1/SCALE so the ACT's
                    # uniform SCALE reproduces reference's scores*SCALE + mask
                    tiles[r].append(
                        np.ascontiguousarray(blks[r].T) * (1.0 / SCALE))
    maskps = [
        np.concatenate(t, axis=0).astype(np.float32) if t
        else np.zeros((128, 256), np.float32) for t in tiles]
    return cats, maskps, n


def _pack_pdim(x, cols):
    """[K, cols] -> [128, K/128 * cols]: kc-chunk-major per partition, so
    each SBUF tile DMA is one contiguous multi-KB chunk per partition row."""
    k = x.shape[0]
    return np.ascontiguousarray(
        x.reshape(k // 128, 128, cols).transpose(1, 0, 2).reshape(128, -1))


def _pack_wq(w):
    """[4096, 4096] -> [128, 131072]: col kcg*16384 + hp*4096 + ki*1024 + c
    so each (kcg, hp) weight tile is one 8KB chunk per partition row."""
    return np.ascontiguousarray(
        w.reshape(8, 4, 128, 4, 1024).transpose(2, 0, 3, 1, 4).reshape(
            128, -1))


def _pack_wo(w):
    """[4096, 4096] -> [128, 131072]: col g*65536 + kc*2048 + e so each
    (g, kc-pair) weight tile is one 8KB chunk per partition row."""
    return np.ascontiguousarray(
        w.reshape(32, 128, 2, 2048).transpose(1, 2, 0, 3).reshape(128, -1))


def _pack_wo8(w):
    """[4096, 4096] -> [2048, 8192]: row 128t+p, col g*4096 + plane*2048
    + c, so each (t, g) load is one contiguous 4KB chunk per row."""
    return np.ascontiguousarray(
        w.reshape(16, 2, 128, 2, 2048).transpose(0, 2, 3, 1, 4).reshape(
            2048, 8192))


def _pack_hs8(x):
    """[4096, 256] -> [128, 8192]: row p, col t*512 + plane*256 + c."""
    return np.ascontiguousarray(
        x.reshape(16, 2, 128, 256).transpose(2, 0, 1, 3).reshape(128, 8192))


def kernel(hidden_states, attention_mask, Wq, Wk, Wv, Wo, trace=False):
    global LAST_RESULTS
    bf = ml_dtypes.bfloat16

    cats, maskps, n_mixed = _classify_mask(attention_mask)
    key = tuple(sorted((k, v if isinstance(v, str) else "m")
                       for k, v in cats.items()))
    if key not in _BUILD_CACHE:
        _BUILD_CACHE[key] = _build(cats, n_mixed)
    nc = _BUILD_CACHE[key]

    # deinterleave rope pairs within each head's 1024 columns
    perm = np.concatenate([np.arange(0, D, 2), np.arange(1, D, 2)])
    cols = np.concatenate([h * D + perm for h in range(NH)])
    wq_p = np.ascontiguousarray(Wq[:, cols])
    wq_bf = _pack_wq(wq_p.astype(bf))
    wk_p = _pack_pdim(np.ascontiguousarray(Wk[:, perm]).astype(bf), D)
    wv_c = _pack_pdim(np.asarray(Wv).astype(bf), D)
    wo_c = _pack_wo(np.asarray(Wo).astype(bf))
    wo8 = _pack_wo8(np.asarray(Wo).astype(NP_F8))

    freqs = 1.0 / (10000.0 ** (np.arange(0, D, 2, dtype=np.float64) / D))
    ang = np.outer(np.arange(S, dtype=np.float64), freqs)  # [S, PD]
    cosT = np.ascontiguousarray(np.cos(ang).T).astype(bf)  # [PD, S]
    sinT = np.ascontiguousarray(np.sin(ang).T).astype(bf)

    hsT = [np.ascontiguousarray(hidden_states[b].T) for b in range(B)]
    hsT_bf = [h.astype(bf) for h in hsT]

    in_maps = []
    for c in range(NCORES):
        b, r = c // 4, c % 4
        g0, g1 = GMAP[r]
        icols = np.r_[256 * g0:256 * (g0 + 1), 256 * g1:256 * (g1 + 1)]
        hs8 = _pack_hs8(
            np.ascontiguousarray(
                hsT[b][:, 256 * g1:256 * (g1 + 1)]).astype(NP_F8))
        in_maps.append({
            "hsq": _pack_pdim(
                np.ascontiguousarray(hsT_bf[b][:, Q * r:Q * (r + 1)]), Q),
            "hsq2": _pack_pdim(
                np.ascontiguousarray(
                    hsT_bf[b][:, 256 * g0:256 * (g0 + 1)]), 256),
            "hs8": hs8,
            "wq": wq_bf,
            "wk": wk_p,
            "wv": wv_c,
            "wo": wo_c,
            "wo8": wo8,
            "cosq": np.ascontiguousarray(cosT[:, Q * r:Q * (r + 1)]),
            "sinq": np.ascontiguousarray(sinT[:, Q * r:Q * (r + 1)]),
            "cosq2": np.ascontiguousarray(cosT[:, icols]),
            "sinq2": np.ascontiguousarray(sinT[:, icols]),
            "maskp": maskps[r],
        })

    res = bass_utils.run_bass_kernel_spmd(
        nc, in_maps, core_ids=list(range(NCORES)), trace=trace)
    LAST_RESULTS = res

    out = np.empty((B, S, H), np.float32)
    for c in range(NCORES):
        b, r = c // 4, c % 4
        g0, g1 = GMAP[r]
        o = res.results[c]["out"]
        out[b, 256 * g0:256 * (g0 + 1), :] = o[0:256]
        out[b, 256 * g1:256 * (g1 + 1), :] = o[256:512]
    return out


# revision 24
# speedup vs baseline: 1.0001x; 1.0001x over previous
"""Multi-head latent attention kernel for Trainium2, 8 NeuronCores.

Problem (hardcoded shapes):
  hidden_states [2, 2048, 4096] f32, attention_mask [1,1,2048,2048] f32,
  Wq [4096,4096], Wk/Wv [4096,1024], Wo [4096,4096].
  4 query heads x 1024 head_dim, 1 kv head, interleaved RoPE, softmax, o-proj.

Sharding: core c = (batch b=c//4, quarter r=c%4), all within-batch groups of 4.
  - k^T / v computed from the core's sequence quarter (hsq input) and
    AllGathered (one combined collective) within the 4-core batch group.
  - Attention is sharded over QUERY positions: core r handles global 256-row
    i-blocks (r, 7-r) for ALL 4 heads, so the output projection is fully
    local. slot0 = block r (rows < 1024), slot1 = block 7-r (rows >= 1024).

Mixed precision: slot0 (early rows, softmax-peaked -> noise-sensitive) runs
entirely in bf16. slot1 (late rows, flat softmax averages out fp8 noise)
uses fp8e4m3 DoubleRow matmuls (2x PE throughput) for the q projection,
probs@V, and the output projection; q@k scores stay bf16 (score noise on
competitive rows is the dominant error channel - validated by numpy sim,
rel err 6.6e-3 vs gate 2e-2). fp8 DoubleRow fuses 2 contraction planes per
instruction: stationary [128,2,M], moving [128,2,N] -> out [M,N]. The host
pre-interleaves wq8/wo8/hs8 into pair-plane layout. Probs are computed as
exp(SCALE*s - 2) to stay under fp8 max 240; the shift cancels in the
softmax normalization (denominator computed from the quantized probs).
"""

import numpy as np
import ml_dtypes

from concourse import bass, mybir, tile, bacc
from concourse import bass_utils

BF16 = mybir.dt.bfloat16
F8 = mybir.dt.float8e4
F32 = mybir.dt.float32
NP_F8 = ml_dtypes.float8_e4m3

B, S, H = 2, 2048, 4096
NH, D = 4, 1024  # query heads, head dim
PD = D // 2  # rope pair count (512)
SCALE = D ** -0.5
PSHIFT = 2.0  # exp bias for fp8 probs (keeps max under fp8e4m3 240)
NCORES = 8
GROUPS = [[0, 1, 2, 3], [4, 5, 6, 7]]

KC = H // 128  # 32 contraction chunks over hidden
DC = D // 128  # 8 d-chunks of head dim
JC = S // 128  # 16 j-chunks (key) of 128
Q = S // 4  # 512, per-core kv sequence quarter

DR = mybir.MatmulPerfMode.DoubleRow

# results of the traced+profiled run (filled by kernel() when trace=True)
LAST_RESULTS = None


def _build(cats, n_mixed):
    """Build the SPMD bass program.

    cats: dict (lb, jc) -> "skip" | "clean" | int (index into packed mask
    tiles); lb in {0,1} is the local 256-row i-block, union over cores.
    lb=0 is the early block (bf16), lb=1 the late block (fp8 path).
    """
    nc = bacc.Bacc("TRN2", target_bir_lowering=False, debug=False,
                   num_devices=NCORES)

    hsq_d = nc.dram_tensor("hsq", [128, KC * Q], BF16, kind="ExternalInput")
    hsq2_d = nc.dram_tensor("hsq2", [128, KC * 256], BF16, kind="ExternalInput")
    hs8_d = nc.dram_tensor("hs8", [128, 16 * 512], F8, kind="ExternalInput")
    wq_d = nc.dram_tensor("wq", [128, KC * H], BF16, kind="ExternalInput")
    wk_d = nc.dram_tensor("wk", [128, KC * D], BF16, kind="ExternalInput")
    wv_d = nc.dram_tensor("wv", [128, KC * D], BF16, kind="ExternalInput")
    wo_d = nc.dram_tensor("wo", [128, KC * H], BF16, kind="ExternalInput")
    wo8_d = nc.dram_tensor("wo8", [2048, 2 * H], F8, kind="ExternalInput")
    cosq_d = nc.dram_tensor("cosq", [PD, Q], BF16, kind="ExternalInput")
    sinq_d = nc.dram_tensor("sinq", [PD, Q], BF16, kind="ExternalInput")
    cosq2_d = nc.dram_tensor("cosq2", [PD, 512], BF16, kind="ExternalInput")
    sinq2_d = nc.dram_tensor("sinq2", [PD, 512], BF16, kind="ExternalInput")
    nmask = max(n_mixed, 1)
    maskp_d = nc.dram_tensor("maskp", [nmask * 128, 256], F32,
                             kind="ExternalInput")
    out_d = nc.dram_tensor("out", [512, H], F32, kind="ExternalOutput")

    # combined k+v collective bounce: rows [0:1024] = kT slice [1024, Q],
    # rows [1024:2048] = v slice [Q, 1024] flattened row-major to [1024, 512]
    # (a single collective: split k/v AllGathers land on a DMA ring that
    # blocks all later input DMAs -- measured 90us+ of pure queue stall)
    kv_in = nc.dram_tensor("kv_in", [2048, 512], BF16, kind="Internal")
    kv_out = nc.dram_tensor("kv_out", [4 * 2048, 512], BF16, kind="Internal")

    PAIRS = [(0, 4), (1, 5), (2, 6), (3, 7)]

    with tile.TileContext(nc) as tc:
        with tc.tile_pool(name="pers", bufs=1) as pers:
            ones_col = pers.tile([128, 1], BF16, name="ones_col",
                                 tag="ones_col")
            nc.vector.memset(ones_col[:], 1.0)
            # DR ldweights needs outer free step 16B-aligned -> pad to 16
            ones8 = pers.tile([128, 2, 16], F8, name="ones8", tag="ones8")
            nc.vector.memset(ones8[:], 1.0)
            ones_row = pers.tile([1, 128], F32, name="ones_row",
                                 tag="ones_row")
            nc.vector.memset(ones_row[:], 1.0)
            ebias = pers.tile([128, 1], F32, name="ebias", tag="ebias")
            nc.vector.memset(ebias[:], -PSHIFT)
            pq = pers
            # q^T per slot: 32 chunks [128 d, 256 i] (4 heads x 8 dc)
            qT0 = [pq.tile([128, 256], BF16, name=f"qt0_{i}",
                           tag=f"qt0_{i}") for i in range(4 * DC)]
            qT1 = [pq.tile([128, 256], BF16, name=f"qt1_{i}",
                           tag=f"qt1_{i}") for i in range(4 * DC)]
            kT = [pq.tile([128, S], BF16, name=f"kt{i}", tag=f"kt{i}")
                  for i in range(DC)]
            # v for slot0 (bf16, first 8 jc only) and slot1 (fp8 jc pairs)
            vT0 = [pq.tile([128, D], BF16, name=f"vt0_{i}", tag=f"vt0_{i}")
                   for i in range(JC // 2)]
            v8 = [pq.tile([128, 2, D], F8, name=f"v8_{i}", tag=f"v8_{i}")
                  for i in range(JC // 2)]
            # mask tiles resident in SBUF, loaded once, shared across heads
            nmt = max(n_mixed, 1)
            mask_t = [pq.tile([128, 256], F32, name=f"mk{i}", tag=f"mk{i}")
                      for i in range(nmt)]

            # ============ phase A1: k/v projections + kv AG ============
            with (
                tc.tile_pool(name="pa1", bufs=3) as pa1,
                tc.tile_pool(name="paps", bufs=8, space="PSUM") as paps,
            ):
                hqc = [pa1.tile([128, 8, Q], BF16, name=f"hqc{i}",
                                tag=f"hqc{i}", bufs=1) for i in range(4)]
                for i in range(4):
                    nc.sync.dma_start(
                        hqc[i][:],
                        hsq_d[:, 4096 * i:4096 * (i + 1)].rearrange(
                            "p (kc s) -> p kc s", kc=8))

                def rope_pair(pool, ps_e, ps_o, c_t, s_t, out_e, out_o, n):
                    """Stage psum pair to bf16, apply rope, write outputs."""
                    st_e = pool.tile([128, n], BF16, name="stg", tag="stg",
                                     bufs=6)
                    st_o = pool.tile([128, n], BF16, name="stg", tag="stg",
                                     bufs=6)
                    nc.scalar.activation(st_e[:], ps_e[:],
                                         mybir.ActivationFunctionType.Copy)
                    nc.scalar.activation(st_o[:], ps_o[:],
                                         mybir.ActivationFunctionType.Copy)
                    t1 = pool.tile([128, n], BF16, name="rtmp", tag="rtmp",
                                   bufs=4)
                    t2 = pool.tile([128, n], BF16, name="rtmp", tag="rtmp",
                                   bufs=4)
                    nc.vector.tensor_mul(t1[:], st_e[:], c_t)
                    nc.vector.tensor_mul(t2[:], st_o[:], s_t)
                    nc.vector.tensor_sub(out_e, t1[:], t2[:])
                    t3 = pool.tile([128, n], BF16, name="rtmp", tag="rtmp",
                                   bufs=4)
                    t4 = pool.tile([128, n], BF16, name="rtmp", tag="rtmp",
                                   bufs=4)
                    nc.vector.tensor_mul(t3[:], st_o[:], c_t)
                    nc.vector.tensor_mul(t4[:], st_e[:], s_t)
                    nc.vector.tensor_add(out_o, t3[:], t4[:])

                # --- K pass: kc-outer over 8 psum banks, then rope pairs ---
                kps = [paps.tile([128, Q], F32, name="mmps", tag="mmps")
                       for _ in range(DC)]
                for kcg in range(KC // 4):
                    wkt = pa1.tile([128, 4, D], BF16, name="wk", tag="wk",
                                   bufs=2)
                    nc.sync.dma_start(
                        wkt[:], wk_d[:, 4096 * kcg:4096 * (kcg + 1)].rearrange(
                            "p (ki c) -> p ki c", ki=4))
                    for ki in range(4):
                        kc = 4 * kcg + ki
                        for dc in range(DC):
                            nc.tensor.matmul(
                                kps[dc][:],
                                wkt[:, ki, 128 * dc:128 * (dc + 1)],
                                hqc[kc // 8][:, kc % 8, :],
                                start=(kc == 0), stop=(kc == KC - 1))
                for pi, (de, do) in enumerate(PAIRS):
                    c_t = pa1.tile([128, Q], BF16, name="ckt", tag="ckt",
                                   bufs=2)
                    s_t = pa1.tile([128, Q], BF16, name="skt", tag="skt",
                                   bufs=2)
                    nc.sync.dma_start(c_t[:],
                                      cosq_d[128 * pi:128 * (pi + 1), :])
                    nc.sync.dma_start(s_t[:],
                                      sinq_d[128 * pi:128 * (pi + 1), :])
                    ke = pa1.tile([128, Q], BF16, name="kout", tag="kout",
                                  bufs=4)
                    ko = pa1.tile([128, Q], BF16, name="kout", tag="kout",
                                  bufs=4)
                    rope_pair(pa1, kps[de], kps[do], c_t[:], s_t[:],
                              ke[:], ko[:], Q)
                    nc.sync.dma_start(kv_in[128 * de:128 * (de + 1), :],
                                      ke[:])
                    nc.sync.dma_start(kv_in[128 * do:128 * (do + 1), :],
                                      ko[:])

                # --- V pass: kc-outer over 8 psum banks ---
                vps = [paps.tile([128, 512], F32, name="mmps", tag="mmps")
                       for _ in range(8)]
                for kcg in range(KC // 4):
                    wvt = pa1.tile([128, 4, D], BF16, name="wv", tag="wv",
                                   bufs=2)
                    nc.sync.dma_start(
                        wvt[:], wv_d[:, 4096 * kcg:4096 * (kcg + 1)].rearrange(
                            "p (ki c) -> p ki c", ki=4))
                    for ki in range(4):
                        kc = 4 * kcg + ki
                        for sc in range(4):
                            for dvb in range(2):
                                nc.tensor.matmul(
                                    vps[sc * 2 + dvb][:],
                                    hqc[kc // 8][:, kc % 8,
                                                 128 * sc:128 * (sc + 1)],
                                    wvt[:, ki, 512 * dvb:512 * (dvb + 1)],
                                    start=(kc == 0), stop=(kc == KC - 1))
                for sc in range(4):
                    for dvb in range(2):
                        vt = pa1.tile([128, 512], BF16, name="vout",
                                      tag="vout", bufs=4)
                        nc.scalar.activation(vt[:], vps[sc * 2 + dvb][:],
                                             mybir.ActivationFunctionType.Copy)
                        dst = kv_in[1024 + 256 * sc:1024 + 256 * (sc + 1), :]
                        dst = dst.rearrange("(p c) f -> p c f", c=2)[:, dvb, :]
                        nc.sync.dma_start(dst, vt[:])

                # --- combined kv AllGather (overlaps q pass) ---
                nc.gpsimd.collective_compute(
                    "AllGather", mybir.AluOpType.bypass, replica_groups=GROUPS,
                    ins=[kv_in.ap().opt()], outs=[kv_out.ap().opt()])
                for i in range(nmt):
                    nc.sync.dma_start(mask_t[i][:],
                                      maskp_d[128 * i:128 * (i + 1), :])

            # ============ phase A2: q projections (slot0 bf16, slot1 fp8) ==
            with (
                tc.tile_pool(name="pa2", bufs=3) as pa2,
                tc.tile_pool(name="paps2", bufs=8, space="PSUM") as paps2,
            ):
                hqc2 = [pa2.tile([128, 8, 256], BF16, name=f"hqc2{i}",
                                 tag=f"hqc2{i}", bufs=1) for i in range(4)]
                for i in range(4):
                    nc.sync.dma_start(
                        hqc2[i][:],
                        hsq2_d[:, 2048 * i:2048 * (i + 1)].rearrange(
                            "p (kc s) -> p kc s", kc=8))
                hs8all = pa2.tile([128, 16, 2, 256], F8, name="hs8all",
                                  tag="hs8all", bufs=1)
                nc.sync.dma_start(
                    hs8all[:],
                    hs8_d.ap().rearrange("p (t i c) -> p t i c", t=16, i=2))
                hs8t = [hs8all[:, t, :, :] for t in range(16)]

                for hp in range(4):
                    # --- slot0 (bf16): 8 dc chunks in 4 psum half-banks ---
                    qps = [paps2.tile([128, 256], F32, name="mmps2",
                                      tag="mmps2") for _ in range(DC)]
                    wq8cs = []
                    for kcg in range(KC // 4):
                        wqt = pa2.tile([128, 4, D], BF16, name="wqs",
                                       tag="wqs", bufs=3)
                        nc.sync.dma_start(
                            wqt[:],
                            wq_d[:, 16384 * kcg + 4096 * hp:
                                 16384 * kcg + 4096 * (hp + 1)].rearrange(
                                "p (ki c) -> p ki c", ki=4))
                        wq8c = pa2.tile([128, 2, 2, D], F8, name="wq8c",
                                        tag="wq8c", bufs=8)
                        for tp in range(2):
                            for pl in range(2):
                                nc.vector.tensor_copy(
                                    wq8c[:, tp, pl, :],
                                    wqt[:, 2 * tp + pl, :])
                        wq8cs.append(wq8c)
                        for ki in range(4):
                            kc = 4 * kcg + ki
                            for dc in range(DC):
                                nc.tensor.matmul(
                                    qps[dc][:],
                                    wqt[:, ki, 128 * dc:128 * (dc + 1)],
                                    hqc2[kc // 8][:, kc % 8, :],
                                    start=(kc == 0), stop=(kc == KC - 1))
                    for pi, (de, do) in enumerate(PAIRS):
                        c_t = pa2.tile([128, 256], BF16, name="cq0",
                                       tag="cq0", bufs=3)
                        s_t = pa2.tile([128, 256], BF16, name="sq0",
                                       tag="sq0", bufs=3)
                        nc.sync.dma_start(
                            c_t[:], cosq2_d[128 * pi:128 * (pi + 1), 0:256])
                        nc.sync.dma_start(
                            s_t[:], sinq2_d[128 * pi:128 * (pi + 1), 0:256])
                        rope_pair(pa2, qps[de], qps[do], c_t[:], s_t[:],
                                  qT0[DC * hp + de][:], qT0[DC * hp + do][:],
                                  256)
                    # --- slot1 (fp8 DoubleRow): weights converted on-chip
                    # from the bf16 wqt tiles by the (otherwise idle) DVE ---
                    qps8 = [paps2.tile([128, 256], F32, name="mmps2",
                                       tag="mmps2") for _ in range(DC)]
                    for t in range(16):
                        kcg, tp = t // 2, t % 2
                        for dc in range(DC):
                            nc.tensor.matmul(
                                qps8[dc][:],
                                wq8cs[kcg][:, tp, :, 128 * dc:128 * (dc + 1)],
                                hs8t[t],
                                start=(t == 0), stop=(t == 15),
                                perf_mode=DR)
                    for pi, (de, do) in enumerate(PAIRS):
                        c_t = pa2.tile([128, 256], BF16, name="cq1",
                                       tag="cq1", bufs=3)
                        s_t = pa2.tile([128, 256], BF16, name="sq1",
                                       tag="sq1", bufs=3)
                        nc.sync.dma_start(
                            c_t[:], cosq2_d[128 * pi:128 * (pi + 1), 256:512])
                        nc.sync.dma_start(
                            s_t[:], sinq2_d[128 * pi:128 * (pi + 1), 256:512])
                        rope_pair(pa2, qps8[de], qps8[do], c_t[:], s_t[:],
                                  qT1[DC * hp + de][:], qT1[DC * hp + do][:],
                                  256)
                    if hp == 2:
                        # kT quarters 0-1: queue reaches here ~3/4 through
                        # the q-pass DMA stream, after the AllGather ends
                        for r in (0, 1):
                            for dc in range(DC):
                                nc.sync.dma_start(
                                    kT[dc][:, Q * r:Q * (r + 1)],
                                    kv_out[2048 * r + 128 * dc:
                                           2048 * r + 128 * (dc + 1), :])
                    if hp == 3:
                        for r in (2, 3):
                            for dc in range(DC):
                                nc.sync.dma_start(
                                    kT[dc][:, Q * r:Q * (r + 1)],
                                    kv_out[2048 * r + 128 * dc:
                                           2048 * r + 128 * (dc + 1), :])

            # --- remaining kT/v loads in attention need-order ---
            with tc.tile_pool(name="vload", bufs=2) as vload:
                def load_v(jc):
                    base = 2048 * (jc // 4) + 1024 + 256 * (jc % 4)
                    vsrc = kv_out[base:base + 256, :].rearrange(
                        "(p c) f -> p (c f)", c=2)
                    if jc < 8:
                        nc.sync.dma_start(vT0[jc][:], vsrc)
                        nc.vector.tensor_copy(v8[jc // 2][:, jc % 2, :],
                                              vT0[jc][:])
                    else:
                        vtmp = vload.tile([128, D], BF16, name="vtmp",
                                          tag="vtmp", bufs=2)
                        nc.sync.dma_start(vtmp[:], vsrc)
                        nc.vector.tensor_copy(v8[jc // 2][:, jc % 2, :],
                                              vtmp[:])

                for jc in range(JC):
                    load_v(jc)

            # ========== phase B+C: attention then o-proj ==========
            # Order: slot0 attn, slot1 attn (DMA-light, lets the wo stream
            # prefetch), then both o-proj passes (DMA-heavy, run exclusive).
            with tc.tile_pool(name="pb", bufs=2) as pb:
                attnT0 = [pb.tile([128, 256], BF16, name=f"at0_{i}",
                                  tag=f"at0_{i}", bufs=1) for i in range(KC)]
                attnT8 = [pb.tile([128, 2, 256], F8, name=f"at8_{i}",
                                  tag=f"at8_{i}", bufs=1)
                          for i in range(KC // 2)]

                live0 = [jc for jc in range(JC) if cats[(0, jc)] != "skip"]
                with tc.tile_pool(name="psa", bufs=2, space="PSUM") as psa:
                    for slot in range(2):
                        for h in range(NH):
                            # ---- scores + exp ----
                            if slot == 0:
                                pT = {}
                                for jc in live0:
                                    sps = psa.tile([128, 256], F32,
                                                   name="sps", tag="sps",
                                                   bufs=3)
                                    for dc in range(DC):
                                        nc.tensor.matmul(
                                            sps[:],
                                            kT[dc][:,
                                                   128 * jc:128 * (jc + 1)],
                                            qT0[DC * h + dc][:],
                                            start=(dc == 0),
                                            stop=(dc == DC - 1))
                                    cat = cats[(0, jc)]
                                    if isinstance(cat, int):
                                        nc.vector.tensor_add(
                                            sps[:], sps[:], mask_t[cat][:])
                                    pt = pb.tile([128, 256], BF16,
                                                 name=f"pt{jc}",
                                                 tag=f"pt{jc}", bufs=2)
                                    nc.scalar.activation(
                                        pt[:], sps[:],
                                        mybir.ActivationFunctionType.Exp,
                                        scale=SCALE)
                                    pT[jc] = pt
                            else:
                                p8 = {}
                                for pr in range(JC // 2):
                                    p8[pr] = pb.tile(
                                        [128, 2, 256], F8, name=f"p8_{pr}",
                                        tag=f"p8_{pr}", bufs=2)
                                for jc in range(JC):
                                    cat = cats[(1, jc)]
                                    if cat == "skip":
                                        nc.vector.memset(
                                            p8[jc // 2][:, jc % 2, :], 0.0)
                                        continue
                                    sps = psa.tile([128, 256], F32,
                                                   name="sps", tag="sps",
                                                   bufs=3)
                                    for dc in range(DC):
                                        nc.tensor.matmul(
                                            sps[:],
                                            kT[dc][:,
                                                   128 * jc:128 * (jc + 1)],
                                            qT1[DC * h + dc][:],
                                            start=(dc == 0),
                                            stop=(dc == DC - 1))
                                    if isinstance(cat, int):
                                        nc.vector.tensor_add(
                                            sps[:], sps[:], mask_t[cat][:])
                                    nc.scalar.activation(
                                        p8[jc // 2][:, jc % 2, :], sps[:],
                                        mybir.ActivationFunctionType.Exp,
                                        scale=SCALE, bias=ebias[:])

                            # ---- PV with interleaved denominator ----
                            r_sb = pb.tile([1, 256], F32, name="rsb",
                                           tag="rsb", bufs=2)
                            rbc = pb.tile([128, 256], F32, name="rbc",
                                          tag="rbc", bufs=2)
                            held = []  # (dc2, pvps) awaiting rbc

                            def emit_pv(dc2):
                                pvps = psa.tile([128, 256], F32, name="pvps",
                                                tag="pvps", bufs=3)
                                if slot == 0:
                                    for n, jc in enumerate(live0):
                                        nc.tensor.matmul(
                                            pvps[:],
                                            vT0[jc][:, 128 * dc2:
                                                    128 * (dc2 + 1)],
                                            pT[jc][:], start=(n == 0),
                                            stop=(n == len(live0) - 1))
                                else:
                                    for pr in range(JC // 2):
                                        nc.tensor.matmul(
                                            pvps[:],
                                            v8[pr][:, :, 128 * dc2:
                                                   128 * (dc2 + 1)],
                                            p8[pr][:], start=(pr == 0),
                                            stop=(pr == JC // 2 - 1),
                                            perf_mode=DR)
                                return pvps

                            def emit_evac(dc2, pvps):
                                c = DC * h + dc2
                                if slot == 0:
                                    nc.vector.tensor_mul(
                                        attnT0[c][:], pvps[:], rbc[:])
                                else:
                                    nc.vector.tensor_mul(
                                        attnT8[c // 2][:, c % 2, :],
                                        pvps[:], rbc[:])

                            for dc2 in range(DC):
                                pvps = emit_pv(dc2)
                                if dc2 == 0:
                                    # denominator after PV0 (exps all done)
                                    l_ps = psa.tile([16, 256], F32,
                                                    name="lps", tag="lps",
                                                    bufs=1)
                                    if slot == 0:
                                        for n, jc in enumerate(live0):
                                            nc.tensor.matmul(
                                                l_ps[0:1, :], ones_col[:],
                                                pT[jc][:], start=(n == 0),
                                                stop=(n == len(live0) - 1))
                                    else:
                                        for pr in range(JC // 2):
                                            nc.tensor.matmul(
                                                l_ps[:], ones8[:],
                                                p8[pr][:], start=(pr == 0),
                                                stop=(pr == JC // 2 - 1),
                                                perf_mode=DR)
                                    nc.vector.reciprocal(r_sb[:],
                                                         l_ps[0:1, :])
                                    held.append((dc2, pvps))
                                elif dc2 == 1:
                                    # r broadcast: reciprocal done during PV1
                                    r_ps = psa.tile([128, 256], F32,
                                                    name="rps", tag="rps",
                                                    bufs=1)
                                    nc.tensor.matmul(r_ps[:], ones_row[:],
                                                     r_sb[:], start=True,
                                                     stop=True)
                                    nc.scalar.activation(
                                        rbc[:], r_ps[:],
                                        mybir.ActivationFunctionType.Copy)
                                    held.append((dc2, pvps))
                                    for d, p in held:
                                        emit_evac(d, p)
                                    held = []
                                else:
                                    emit_evac(dc2, pvps)

                # ---- o-proj slot0 (bf16) then slot1 (fp8 DR) ----
                # eb-groups of 4 share one [128, 2048] weight tile so each
                # DMA moves 2-4KB per partition row (descriptor-rate bound)
                with (
                    tc.tile_pool(name="pc", bufs=2) as pc,
                    tc.tile_pool(name="psc", bufs=2, space="PSUM") as psc,
                ):
                    for g in range(2):
                        ops = [psc.tile([128, 512], F32, name="ops",
                                        tag="ops", bufs=8)
                               for _ in range(8)]
                        for t in range(KC // 2):
                            wot = pc.tile([128, 2, 2048], BF16, name="wot",
                                          tag="wot", bufs=3)
                            nc.sync.dma_start(
                                wot[:],
                                wo_d[:, 65536 * g + 4096 * t:
                                     65536 * g + 4096 * (t + 1)].rearrange(
                                    "p (ki c) -> p ki c", ki=2))
                            for ki in range(2):
                                kc = 2 * t + ki
                                for e in range(4):
                                    for ic in range(2):
                                        nc.tensor.matmul(
                                            ops[2 * e + ic][:],
                                            attnT0[kc][:, 128 * ic:
                                                       128 * (ic + 1)],
                                            wot[:, ki, 512 * e:512 * (e + 1)],
                                            start=(kc == 0),
                                            stop=(kc == KC - 1))
                        for ic in range(2):
                            ot = pc.tile([128, 2048], F32, name="ot0",
                                         tag="ot", bufs=2)
                            for e in range(4):
                                nc.vector.tensor_copy(
                                    ot[:, 512 * e:512 * (e + 1)],
                                    ops[2 * e + ic][:])
                            nc.sync.dma_start(
                                out_d[128 * ic:128 * (ic + 1),
                                      2048 * g:2048 * (g + 1)], ot[:])
                    for g in range(2):
                        ops = [psc.tile([128, 512], F32, name="ops",
                                        tag="ops", bufs=8)
                               for _ in range(8)]
                        for t in range(KC // 2):
                            w8g = pc.tile([128, 2, 2048], F8, name="w8g",
                                          tag="w8g", bufs=2)
                            nc.scalar.dma_start(
                                w8g[:],
                                wo8_d[128 * t:128 * (t + 1),
                                      4096 * g:4096 * (g + 1)].rearrange(
                                    "p (i c) -> p i c", i=2))
                            for e in range(4):
                                for ic in range(2):
                                    nc.tensor.matmul(
                                        ops[2 * e + ic][:],
                                        attnT8[t][:, :,
                                                  128 * ic:128 * (ic + 1)],
                                        w8g[:, :, 512 * e:512 * (e + 1)],
                                        start=(t == 0),
                                        stop=(t == KC // 2 - 1),
                                        perf_mode=DR)
                        for ic in range(2):
                            ot = pc.tile([128, 2048], F32, name="ot1",
                                         tag="ot", bufs=2)
                            for e in range(4):
                                nc.vector.tensor_copy(
                                    ot[:, 512 * e:512 * (e + 1)],
                                    ops[2 * e + ic][:])
                            nc.sync.dma_start(
                                out_d[256 + 128 * ic:256 + 128 * (ic + 1),
                                      2048 * g:2048 * (g + 1)], ot[:])

    nc.compile()
    return nc


_BUILD_CACHE = {}

# core r (within its batch group) handles global 256-row i-blocks (r, 7-r)
GMAP = [(r, 7 - r) for r in range(4)]


def _classify_mask(mask):
    """Union-classify each (local block lb, jc) over the 4 quarter cores.

    Returns (cats, per-core packed mask tile arrays, n_mixed). The program
    structure (cats) is shared by all cores; mask tiles are per-core data.
    """
    m = np.asarray(mask).reshape(S, S)  # [i, j]
    cats = {}
    tiles = [[] for _ in range(4)]
    n = 0
    for lb in range(2):
        for jc in range(JC):
            blks = [m[256 * GMAP[r][lb]:256 * (GMAP[r][lb] + 1),
                      128 * jc:128 * (jc + 1)] for r in range(4)]
            if all(np.all(b <= -1e8) for b in blks):
                cats[(lb, jc)] = "skip"
            elif not any(b.any() for b in blks):
                cats[(lb, jc)] = "clean"
            else:
                cats[(lb, jc)] = n
                n += 1
                for r in range(4):
                    # [j, i] orientation, prescaled by 1/SCALE so the ACT's
                    # uniform SCALE reproduces reference's scores*SCALE + mask
                    tiles[r].append(
                        np.ascontiguousarray(blks[r].T) * (1.0 / SCALE))
    maskps = [
        np.concatenate(t, axis=0).astype(np.float32) if t
        else np.zeros((128, 256), np.float32) for t in tiles]
    return cats, maskps, n


def _pack_pdim(x, cols):
    """[K, cols] -> [128, K/128 * cols]: kc-chunk-major per partition, so
    each SBUF tile DMA is one contiguous multi-KB chunk per partition row."""
    k = x.shape[0]
    return np.ascontiguousarray(
        x.reshape(k // 128, 128, cols).transpose(1, 0, 2).reshape(128, -1))


def _pack_wq(w):
    """[4096, 4096] -> [128, 131072]: col kcg*16384 + hp*4096 + ki*1024 + c
    so each (kcg, hp) weight tile is one 8KB chunk per partition row."""
    return np.ascontiguousarray(
        w.reshape(8, 4, 128, 4, 1024).transpose(2, 0, 3, 1, 4).reshape(
            128, -1))


def _pack_wo(w):
    """[4096, 4096] -> [128, 131072]: col g*65536 + kc*2048 + e so each
    (g, kc-pair) weight tile is one 8KB chunk per partition row."""
    return np.ascontiguousarray(
        w.reshape(32, 128, 2, 2048).transpose(1, 2, 0, 3).reshape(128, -1))


def _pack_wo8(w):
    """[4096, 4096] -> [2048, 8192]: row 128t+p, col g*4096 + plane*2048
    + c, so each (t, g) load is one contiguous 4KB chunk per row."""
    return np.ascontiguousarray(
        w.reshape(16, 2, 128, 2, 2048).transpose(0, 2, 3, 1, 4).reshape(
            2048, 8192))


def _pack_hs8(x):
    """[4096, 256] -> [128, 8192]: row p, col t*512 + plane*256 + c."""
    return np.ascontiguousarray(
        x.reshape(16, 2, 128, 256).transpose(2, 0, 1, 3).reshape(128, 8192))


def kernel(hidden_states, attention_mask, Wq, Wk, Wv, Wo, trace=False):
    global LAST_RESULTS
    bf = ml_dtypes.bfloat16

    cats, maskps, n_mixed = _classify_mask(attention_mask)
    key = tuple(sorted((k, v if isinstance(v, str) else "m")
                       for k, v in cats.items()))
    if key not in _BUILD_CACHE:
        _BUILD_CACHE[key] = _build(cats, n_mixed)
    nc = _BUILD_CACHE[key]

    # deinterleave rope pairs within each head's 1024 columns
    perm = np.concatenate([np.arange(0, D, 2), np.arange(1, D, 2)])
    cols = np.concatenate([h * D + perm for h in range(NH)])
    wq_p = np.ascontiguousarray(Wq[:, cols])
    wq_bf = _pack_wq(wq_p.astype(bf))
    wk_p = _pack_pdim(np.ascontiguousarray(Wk[:, perm]).astype(bf), D)
    wv_c = _pack_pdim(np.asarray(Wv).astype(bf), D)
    wo_c = _pack_wo(np.asarray(Wo).astype(bf))
    wo8 = _pack_wo8(np.asarray(Wo).astype(NP_F8))

    freqs = 1.0 / (10000.0 ** (np.arange(0, D, 2, dtype=np.float64) / D))
    ang = np.outer(np.arange(S, dtype=np.float64), freqs)  # [S, PD]
    cosT = np.ascontiguousarray(np.cos(ang).T).astype(bf)  # [PD, S]
    sinT = np.ascontiguousarray(np.sin(ang).T).astype(bf)

    hsT = [np.ascontiguousarray(hidden_states[b].T) for b in range(B)]
    hsT_bf = [h.astype(bf) for h in hsT]

    in_maps = []
    for c in range(NCORES):
        b, r = c // 4, c % 4
        g0, g1 = GMAP[r]
        icols = np.r_[256 * g0:256 * (g0 + 1), 256 * g1:256 * (g1 + 1)]
        hs8 = _pack_hs8(
            np.ascontiguousarray(
                hsT[b][:, 256 * g1:256 * (g1 + 1)]).astype(NP_F8))
        in_maps.append({
            "hsq": _pack_pdim(
                np.ascontiguousarray(hsT_bf[b][:, Q * r:Q * (r + 1)]), Q),
            "hsq2": _pack_pdim(
                np.ascontiguousarray(
                    hsT_bf[b][:, 256 * g0:256 * (g0 + 1)]), 256),
            "hs8": hs8,
            "wq": wq_bf,
            "wk": wk_p,
            "wv": wv_c,
            "wo": wo_c,
            "wo8": wo8,
            "cosq": np.ascontiguousarray(cosT[:, Q * r:Q * (r + 1)]),
            "sinq": np.ascontiguousarray(sinT[:, Q * r:Q * (r + 1)]),
            "cosq2": np.ascontiguousarray(cosT[:, icols]),
            "sinq2": np.ascontiguousarray(sinT[:, icols]),
            "maskp": maskps[r],
        })

    res = bass_utils.run_bass_kernel_spmd(
        nc, in_maps, core_ids=list(range(NCORES)), trace=trace)
    LAST_RESULTS = res

    out = np.empty((B, S, H), np.float32)
    for c in range(NCORES):
        b, r = c // 4, c % 4
        g0, g1 = GMAP[r]
        o = res.results[c]["out"]
        out[b, 256 * g0:256 * (g0 + 1), :] = o[0:256]
        out[b, 256 * g1:256 * (g1 + 1), :] = o[256:512]
    return out


# revision 25
# speedup vs baseline: 1.0096x; 1.0095x over previous
"""Multi-head latent attention kernel for Trainium2, 8 NeuronCores.

Problem (hardcoded shapes):
  hidden_states [2, 2048, 4096] f32, attention_mask [1,1,2048,2048] f32,
  Wq [4096,4096], Wk/Wv [4096,1024], Wo [4096,4096].
  4 query heads x 1024 head_dim, 1 kv head, interleaved RoPE, softmax, o-proj.

Sharding: core c = (batch b=c//4, quarter r=c%4), all within-batch groups of 4.
  - k^T / v computed from the core's sequence quarter (hsq input) and
    AllGathered (one combined collective) within the 4-core batch group.
  - Attention is sharded over QUERY positions: core r handles global 256-row
    i-blocks (r, 7-r) for ALL 4 heads, so the output projection is fully
    local. slot0 = block r (rows < 1024), slot1 = block 7-r (rows >= 1024).

Mixed precision: slot0 (early rows, softmax-peaked -> noise-sensitive) runs
entirely in bf16. slot1 (late rows, flat softmax averages out fp8 noise)
uses fp8e4m3 DoubleRow matmuls (2x PE throughput) for the q projection,
probs@V, and the output projection; q@k scores stay bf16 (score noise on
competitive rows is the dominant error channel - validated by numpy sim,
rel err 6.6e-3 vs gate 2e-2). fp8 DoubleRow fuses 2 contraction planes per
instruction: stationary [128,2,M], moving [128,2,N] -> out [M,N]. The host
pre-interleaves wq8/wo8/hs8 into pair-plane layout. Probs are computed as
exp(SCALE*s - 2) to stay under fp8 max 240; the shift cancels in the
softmax normalization (denominator computed from the quantized probs).
"""

import numpy as np
import ml_dtypes

from concourse import bass, mybir, tile, bacc
from concourse import bass_utils

BF16 = mybir.dt.bfloat16
F8 = mybir.dt.float8e4
F32 = mybir.dt.float32
NP_F8 = ml_dtypes.float8_e4m3

B, S, H = 2, 2048, 4096
NH, D = 4, 1024  # query heads, head dim
PD = D // 2  # rope pair count (512)
SCALE = D ** -0.5
PSHIFT = 2.0  # exp bias for fp8 probs (keeps max under fp8e4m3 240)
NCORES = 8
GROUPS = [[0, 1, 2, 3], [4, 5, 6, 7]]

KC = H // 128  # 32 contraction chunks over hidden
DC = D // 128  # 8 d-chunks of head dim
JC = S // 128  # 16 j-chunks (key) of 128
Q = S // 4  # 512, per-core kv sequence quarter

DR = mybir.MatmulPerfMode.DoubleRow

# results of the traced+profiled run (filled by kernel() when trace=True)
LAST_RESULTS = None


def _build(cats, n_mixed):
    """Build the SPMD bass program.

    cats: dict (lb, jc) -> "skip" | "clean" | int (index into packed mask
    tiles); lb in {0,1} is the local 256-row i-block, union over cores.
    lb=0 is the early block (bf16), lb=1 the late block (fp8 path).
    """
    nc = bacc.Bacc("TRN2", target_bir_lowering=False, debug=False,
                   num_devices=NCORES)

    hsq_d = nc.dram_tensor("hsq", [128, KC * Q], BF16, kind="ExternalInput")
    hsq2_d = nc.dram_tensor("hsq2", [128, KC * 256], BF16, kind="ExternalInput")
    hs8_d = nc.dram_tensor("hs8", [128, 16 * 512], F8, kind="ExternalInput")
    wq_d = nc.dram_tensor("wq", [128, KC * H], BF16, kind="ExternalInput")
    wk_d = nc.dram_tensor("wk", [128, KC * D], BF16, kind="ExternalInput")
    wv_d = nc.dram_tensor("wv", [128, KC * D], BF16, kind="ExternalInput")
    wo_d = nc.dram_tensor("wo", [128, KC * H], BF16, kind="ExternalInput")
    wo8_d = nc.dram_tensor("wo8", [2048, 2 * H], F8, kind="ExternalInput")
    cosq_d = nc.dram_tensor("cosq", [PD, Q], BF16, kind="ExternalInput")
    sinq_d = nc.dram_tensor("sinq", [PD, Q], BF16, kind="ExternalInput")
    cosq2_d = nc.dram_tensor("cosq2", [PD, 512], BF16, kind="ExternalInput")
    sinq2_d = nc.dram_tensor("sinq2", [PD, 512], BF16, kind="ExternalInput")
    nmask = max(n_mixed, 1)
    maskp_d = nc.dram_tensor("maskp", [nmask * 128, 256], F32,
                             kind="ExternalInput")
    out_d = nc.dram_tensor("out", [512, H], F32, kind="ExternalOutput")

    # combined k+v collective bounce: rows [0:1024] = kT slice [1024, Q],
    # rows [1024:2048] = v slice [Q, 1024] flattened row-major to [1024, 512]
    # (a single collective: split k/v AllGathers land on a DMA ring that
    # blocks all later input DMAs -- measured 90us+ of pure queue stall)
    kv_in = nc.dram_tensor("kv_in", [2048, 512], BF16, kind="Internal")
    kv_out = nc.dram_tensor("kv_out", [4 * 2048, 512], BF16, kind="Internal")

    PAIRS = [(0, 4), (1, 5), (2, 6), (3, 7)]

    with tile.TileContext(nc) as tc:
        with tc.tile_pool(name="pers", bufs=1) as pers:
            ones_col = pers.tile([128, 1], BF16, name="ones_col",
                                 tag="ones_col")
            nc.vector.memset(ones_col[:], 1.0)
            # DR ldweights needs outer free step 16B-aligned -> pad to 16
            ones8 = pers.tile([128, 2, 16], F8, name="ones8", tag="ones8")
            nc.vector.memset(ones8[:], 1.0)
            ones_row = pers.tile([1, 128], F32, name="ones_row",
                                 tag="ones_row")
            nc.vector.memset(ones_row[:], 1.0)
            ebias = pers.tile([128, 1], F32, name="ebias", tag="ebias")
            nc.vector.memset(ebias[:], -PSHIFT)
            pq = pers
            # q^T per slot: 32 chunks [128 d, 256 i] (4 heads x 8 dc)
            qT0 = [pq.tile([128, 256], BF16, name=f"qt0_{i}",
                           tag=f"qt0_{i}") for i in range(4 * DC)]
            qT1 = [pq.tile([128, 256], BF16, name=f"qt1_{i}",
                           tag=f"qt1_{i}") for i in range(4 * DC)]
            kT = [pq.tile([128, S], BF16, name=f"kt{i}", tag=f"kt{i}")
                  for i in range(DC)]
            # v for slot0 (bf16, first 8 jc only) and slot1 (fp8 jc pairs)
            vT0 = [pq.tile([128, D], BF16, name=f"vt0_{i}", tag=f"vt0_{i}")
                   for i in range(JC // 2)]
            v8 = [pq.tile([128, 2, D], F8, name=f"v8_{i}", tag=f"v8_{i}")
                  for i in range(JC // 2)]
            # mask tiles resident in SBUF, loaded once, shared across heads
            nmt = max(n_mixed, 1)
            mask_t = [pq.tile([128, 256], F32, name=f"mk{i}", tag=f"mk{i}")
                      for i in range(nmt)]

            # ============ phase A1: k/v projections + kv AG ============
            with (
                tc.tile_pool(name="pa1", bufs=3) as pa1,
                tc.tile_pool(name="paps", bufs=8, space="PSUM") as paps,
            ):
                hqc = [pa1.tile([128, 8, Q], BF16, name=f"hqc{i}",
                                tag=f"hqc{i}", bufs=1) for i in range(4)]
                for i in range(4):
                    nc.sync.dma_start(
                        hqc[i][:],
                        hsq_d[:, 4096 * i:4096 * (i + 1)].rearrange(
                            "p (kc s) -> p kc s", kc=8))

                def rope_pair(pool, ps_e, ps_o, c_t, s_t, out_e, out_o, n):
                    """Stage psum pair to bf16, apply rope, write outputs."""
                    st_e = pool.tile([128, n], BF16, name="stg", tag="stg",
                                     bufs=6)
                    st_o = pool.tile([128, n], BF16, name="stg", tag="stg",
                                     bufs=6)
                    nc.scalar.activation(st_e[:], ps_e[:],
                                         mybir.ActivationFunctionType.Copy)
                    nc.scalar.activation(st_o[:], ps_o[:],
                                         mybir.ActivationFunctionType.Copy)
                    t1 = pool.tile([128, n], BF16, name="rtmp", tag="rtmp",
                                   bufs=4)
                    t2 = pool.tile([128, n], BF16, name="rtmp", tag="rtmp",
                                   bufs=4)
                    nc.vector.tensor_mul(t1[:], st_e[:], c_t)
                    nc.vector.tensor_mul(t2[:], st_o[:], s_t)
                    nc.vector.tensor_sub(out_e, t1[:], t2[:])
                    t3 = pool.tile([128, n], BF16, name="rtmp", tag="rtmp",
                                   bufs=4)
                    t4 = pool.tile([128, n], BF16, name="rtmp", tag="rtmp",
                                   bufs=4)
                    nc.vector.tensor_mul(t3[:], st_o[:], c_t)
                    nc.vector.tensor_mul(t4[:], st_e[:], s_t)
                    nc.vector.tensor_add(out_o, t3[:], t4[:])

                # --- K pass: kc-outer over 8 psum banks, then rope pairs ---
                kps = [paps.tile([128, Q], F32, name="mmps", tag="mmps")
                       for _ in range(DC)]
                for kcg in range(KC // 4):
                    wkt = pa1.tile([128, 4, D], BF16, name="wk", tag="wk",
                                   bufs=2)
                    nc.sync.dma_start(
                        wkt[:], wk_d[:, 4096 * kcg:4096 * (kcg + 1)].rearrange(
                            "p (ki c) -> p ki c", ki=4))
                    for ki in range(4):
                        kc = 4 * kcg + ki
                        for dc in range(DC):
                            nc.tensor.matmul(
                                kps[dc][:],
                                wkt[:, ki, 128 * dc:128 * (dc + 1)],
                                hqc[kc // 8][:, kc % 8, :],
                                start=(kc == 0), stop=(kc == KC - 1))
                for pi, (de, do) in enumerate(PAIRS):
                    c_t = pa1.tile([128, Q], BF16, name="ckt", tag="ckt",
                                   bufs=2)
                    s_t = pa1.tile([128, Q], BF16, name="skt", tag="skt",
                                   bufs=2)
                    nc.sync.dma_start(c_t[:],
                                      cosq_d[128 * pi:128 * (pi + 1), :])
                    nc.sync.dma_start(s_t[:],
                                      sinq_d[128 * pi:128 * (pi + 1), :])
                    ke = pa1.tile([128, Q], BF16, name="kout", tag="kout",
                                  bufs=4)
                    ko = pa1.tile([128, Q], BF16, name="kout", tag="kout",
                                  bufs=4)
                    rope_pair(pa1, kps[de], kps[do], c_t[:], s_t[:],
                              ke[:], ko[:], Q)
                    nc.sync.dma_start(kv_in[128 * de:128 * (de + 1), :],
                                      ke[:])
                    nc.sync.dma_start(kv_in[128 * do:128 * (do + 1), :],
                                      ko[:])

                # --- V pass: kc-outer over 8 psum banks ---
                vps = [paps.tile([128, 512], F32, name="mmps", tag="mmps")
                       for _ in range(8)]
                for kcg in range(KC // 4):
                    wvt = pa1.tile([128, 4, D], BF16, name="wv", tag="wv",
                                   bufs=2)
                    nc.sync.dma_start(
                        wvt[:], wv_d[:, 4096 * kcg:4096 * (kcg + 1)].rearrange(
                            "p (ki c) -> p ki c", ki=4))
                    for ki in range(4):
                        kc = 4 * kcg + ki
                        for sc in range(4):
                            for dvb in range(2):
                                nc.tensor.matmul(
                                    vps[sc * 2 + dvb][:],
                                    hqc[kc // 8][:, kc % 8,
                                                 128 * sc:128 * (sc + 1)],
                                    wvt[:, ki, 512 * dvb:512 * (dvb + 1)],
                                    start=(kc == 0), stop=(kc == KC - 1))
                for sc in range(4):
                    for dvb in range(2):
                        vt = pa1.tile([128, 512], BF16, name="vout",
                                      tag="vout", bufs=4)
                        nc.scalar.activation(vt[:], vps[sc * 2 + dvb][:],
                                             mybir.ActivationFunctionType.Copy)
                        dst = kv_in[1024 + 256 * sc:1024 + 256 * (sc + 1), :]
                        dst = dst.rearrange("(p c) f -> p c f", c=2)[:, dvb, :]
                        nc.sync.dma_start(dst, vt[:])

                # --- combined kv AllGather (overlaps q pass) ---
                nc.gpsimd.collective_compute(
                    "AllGather", mybir.AluOpType.bypass, replica_groups=GROUPS,
                    ins=[kv_in.ap().opt()], outs=[kv_out.ap().opt()])
                for i in range(nmt):
                    nc.sync.dma_start(mask_t[i][:],
                                      maskp_d[128 * i:128 * (i + 1), :])

            # ============ phase A2: q projections (slot0 bf16, slot1 fp8) ==
            with (
                tc.tile_pool(name="pa2", bufs=3) as pa2,
                tc.tile_pool(name="paps2", bufs=8, space="PSUM") as paps2,
            ):
                hqc2 = [pa2.tile([128, 8, 256], BF16, name=f"hqc2{i}",
                                 tag=f"hqc2{i}", bufs=1) for i in range(4)]
                for i in range(4):
                    nc.sync.dma_start(
                        hqc2[i][:],
                        hsq2_d[:, 2048 * i:2048 * (i + 1)].rearrange(
                            "p (kc s) -> p kc s", kc=8))
                hs8all = pa2.tile([128, 16, 2, 256], F8, name="hs8all",
                                  tag="hs8all", bufs=1)
                nc.sync.dma_start(
                    hs8all[:],
                    hs8_d.ap().rearrange("p (t i c) -> p t i c", t=16, i=2))
                hs8t = [hs8all[:, t, :, :] for t in range(16)]

                for hp in range(4):
                    # --- slot0 (bf16): 8 dc chunks in 4 psum half-banks ---
                    qps = [paps2.tile([128, 256], F32, name="mmps2",
                                      tag="mmps2") for _ in range(DC)]
                    wq8cs = []
                    for kcg in range(KC // 4):
                        wqt = pa2.tile([128, 4, D], BF16, name="wqs",
                                       tag="wqs", bufs=3)
                        nc.sync.dma_start(
                            wqt[:],
                            wq_d[:, 16384 * kcg + 4096 * hp:
                                 16384 * kcg + 4096 * (hp + 1)].rearrange(
                                "p (ki c) -> p ki c", ki=4))
                        wq8c = pa2.tile([128, 2, 2, D], F8, name="wq8c",
                                        tag="wq8c", bufs=8)
                        for tp in range(2):
                            for pl in range(2):
                                nc.vector.tensor_copy(
                                    wq8c[:, tp, pl, :],
                                    wqt[:, 2 * tp + pl, :])
                        wq8cs.append(wq8c)
                        for ki in range(4):
                            kc = 4 * kcg + ki
                            for dc in range(DC):
                                nc.tensor.matmul(
                                    qps[dc][:],
                                    wqt[:, ki, 128 * dc:128 * (dc + 1)],
                                    hqc2[kc // 8][:, kc % 8, :],
                                    start=(kc == 0), stop=(kc == KC - 1))
                    for pi, (de, do) in enumerate(PAIRS):
                        c_t = pa2.tile([128, 256], BF16, name="cq0",
                                       tag="cq0", bufs=3)
                        s_t = pa2.tile([128, 256], BF16, name="sq0",
                                       tag="sq0", bufs=3)
                        nc.sync.dma_start(
                            c_t[:], cosq2_d[128 * pi:128 * (pi + 1), 0:256])
                        nc.sync.dma_start(
                            s_t[:], sinq2_d[128 * pi:128 * (pi + 1), 0:256])
                        rope_pair(pa2, qps[de], qps[do], c_t[:], s_t[:],
                                  qT0[DC * hp + de][:], qT0[DC * hp + do][:],
                                  256)
                    # --- slot1 (fp8 DoubleRow): weights converted on-chip
                    # from the bf16 wqt tiles by the (otherwise idle) DVE ---
                    qps8 = [paps2.tile([128, 256], F32, name="mmps2",
                                       tag="mmps2") for _ in range(DC)]
                    for t in range(16):
                        kcg, tp = t // 2, t % 2
                        for dc in range(DC):
                            nc.tensor.matmul(
                                qps8[dc][:],
                                wq8cs[kcg][:, tp, :, 128 * dc:128 * (dc + 1)],
                                hs8t[t],
                                start=(t == 0), stop=(t == 15),
                                perf_mode=DR)
                    for pi, (de, do) in enumerate(PAIRS):
                        c_t = pa2.tile([128, 256], BF16, name="cq1",
                                       tag="cq1", bufs=3)
                        s_t = pa2.tile([128, 256], BF16, name="sq1",
                                       tag="sq1", bufs=3)
                        nc.sync.dma_start(
                            c_t[:], cosq2_d[128 * pi:128 * (pi + 1), 256:512])
                        nc.sync.dma_start(
                            s_t[:], sinq2_d[128 * pi:128 * (pi + 1), 256:512])
                        rope_pair(pa2, qps8[de], qps8[do], c_t[:], s_t[:],
                                  qT1[DC * hp + de][:], qT1[DC * hp + do][:],
                                  256)
                    if hp == 2:
                        # kT quarters 0-1: queue reaches here ~3/4 through
                        # the q-pass DMA stream, after the AllGather ends
                        for r in (0, 1):
                            for dc in range(DC):
                                nc.sync.dma_start(
                                    kT[dc][:, Q * r:Q * (r + 1)],
                                    kv_out[2048 * r + 128 * dc:
                                           2048 * r + 128 * (dc + 1), :])
                    if hp == 3:
                        for r in (2, 3):
                            for dc in range(DC):
                                nc.sync.dma_start(
                                    kT[dc][:, Q * r:Q * (r + 1)],
                                    kv_out[2048 * r + 128 * dc:
                                           2048 * r + 128 * (dc + 1), :])

            # --- remaining kT/v loads in attention need-order ---
            with tc.tile_pool(name="vload", bufs=2) as vload:
                def load_v(jc):
                    base = 2048 * (jc // 4) + 1024 + 256 * (jc % 4)
                    vsrc = kv_out[base:base + 256, :].rearrange(
                        "(p c) f -> p (c f)", c=2)
                    if jc < 8:
                        nc.sync.dma_start(vT0[jc][:], vsrc)
                        nc.vector.tensor_copy(v8[jc // 2][:, jc % 2, :],
                                              vT0[jc][:])
                    else:
                        vtmp = vload.tile([128, D], BF16, name="vtmp",
                                          tag="vtmp", bufs=2)
                        nc.sync.dma_start(vtmp[:], vsrc)
                        nc.vector.tensor_copy(v8[jc // 2][:, jc % 2, :],
                                              vtmp[:])

                for jc in range(JC):
                    load_v(jc)

            # ========== phase B+C: attention then o-proj ==========
            # Order: slot0 attn, slot1 attn (DMA-light, lets the wo stream
            # prefetch), then both o-proj passes (DMA-heavy, run exclusive).
            with tc.tile_pool(name="pb", bufs=2) as pb:
                attnT0 = [pb.tile([128, 256], BF16, name=f"at0_{i}",
                                  tag=f"at0_{i}", bufs=1) for i in range(KC)]
                attnT8 = [pb.tile([128, 2, 256], F8, name=f"at8_{i}",
                                  tag=f"at8_{i}", bufs=1)
                          for i in range(KC // 2)]

                live0 = [jc for jc in range(JC) if cats[(0, jc)] != "skip"]
                with tc.tile_pool(name="psa", bufs=2, space="PSUM") as psa:
                    for slot in range(2):
                        for h in range(NH):
                            # ---- scores + exp ----
                            if slot == 0:
                                pT = {}
                                for jc in live0:
                                    sps = psa.tile([128, 256], F32,
                                                   name="sps", tag="sps",
                                                   bufs=3)
                                    for dc in range(DC):
                                        nc.tensor.matmul(
                                            sps[:],
                                            kT[dc][:,
                                                   128 * jc:128 * (jc + 1)],
                                            qT0[DC * h + dc][:],
                                            start=(dc == 0),
                                            stop=(dc == DC - 1))
                                    cat = cats[(0, jc)]
                                    if isinstance(cat, int):
                                        nc.vector.tensor_add(
                                            sps[:], sps[:], mask_t[cat][:])
                                    pt = pb.tile([128, 256], BF16,
                                                 name=f"pt{jc}",
                                                 tag=f"pt{jc}", bufs=2)
                                    nc.scalar.activation(
                                        pt[:], sps[:],
                                        mybir.ActivationFunctionType.Exp,
                                        scale=SCALE)
                                    pT[jc] = pt
                            else:
                                p8 = {}
                                for pr in range(JC // 2):
                                    p8[pr] = pb.tile(
                                        [128, 2, 256], F8, name=f"p8_{pr}",
                                        tag=f"p8_{pr}", bufs=2)
                                for jc in range(JC):
                                    cat = cats[(1, jc)]
                                    if cat == "skip":
                                        nc.vector.memset(
                                            p8[jc // 2][:, jc % 2, :], 0.0)
                                        continue
                                    sps = psa.tile([128, 256], F32,
                                                   name="sps", tag="sps",
                                                   bufs=3)
                                    for dc in range(DC):
                                        nc.tensor.matmul(
                                            sps[:],
                                            kT[dc][:,
                                                   128 * jc:128 * (jc + 1)],
                                            qT1[DC * h + dc][:],
                                            start=(dc == 0),
                                            stop=(dc == DC - 1))
                                    if isinstance(cat, int):
                                        nc.vector.tensor_add(
                                            sps[:], sps[:], mask_t[cat][:])
                                    nc.scalar.activation(
                                        p8[jc // 2][:, jc % 2, :], sps[:],
                                        mybir.ActivationFunctionType.Exp,
                                        scale=SCALE, bias=ebias[:])

                            # ---- PV with interleaved denominator ----
                            r_sb = pb.tile([1, 256], F32, name="rsb",
                                           tag="rsb", bufs=2)
                            rbc = pb.tile([128, 256], F32, name="rbc",
                                          tag="rbc", bufs=2)
                            held = []  # (dc2, pvps) awaiting rbc

                            def emit_pv(dc2):
                                pvps = psa.tile([128, 256], F32, name="pvps",
                                                tag="pvps", bufs=3)
                                if slot == 0:
                                    for n, jc in enumerate(live0):
                                        nc.tensor.matmul(
                                            pvps[:],
                                            vT0[jc][:, 128 * dc2:
                                                    128 * (dc2 + 1)],
                                            pT[jc][:], start=(n == 0),
                                            stop=(n == len(live0) - 1))
                                else:
                                    for pr in range(JC // 2):
                                        nc.tensor.matmul(
                                            pvps[:],
                                            v8[pr][:, :, 128 * dc2:
                                                   128 * (dc2 + 1)],
                                            p8[pr][:], start=(pr == 0),
                                            stop=(pr == JC // 2 - 1),
                                            perf_mode=DR)
                                return pvps

                            def emit_evac(dc2, pvps):
                                c = DC * h + dc2
                                if slot == 0:
                                    nc.vector.tensor_mul(
                                        attnT0[c][:], pvps[:], rbc[:])
                                else:
                                    nc.vector.tensor_mul(
                                        attnT8[c // 2][:, c % 2, :],
                                        pvps[:], rbc[:])

                            for dc2 in range(DC):
                                pvps = emit_pv(dc2)
                                if dc2 == 0:
                                    # denominator after PV0 (exps all done)
                                    l_ps = psa.tile([16, 256], F32,
                                                    name="lps", tag="lps",
                                                    bufs=1)
                                    if slot == 0:
                                        for n, jc in enumerate(live0):
                                            nc.tensor.matmul(
                                                l_ps[0:1, :], ones_col[:],
                                                pT[jc][:], start=(n == 0),
                                                stop=(n == len(live0) - 1))
                                    else:
                                        for pr in range(JC // 2):
                                            nc.tensor.matmul(
                                                l_ps[:], ones8[:],
                                                p8[pr][:], start=(pr == 0),
                                                stop=(pr == JC // 2 - 1),
                                                perf_mode=DR)
                                    nc.vector.reciprocal(r_sb[:],
                                                         l_ps[0:1, :])
                                    held.append((dc2, pvps))
                                elif dc2 == 1:
                                    # r broadcast: reciprocal done during PV1
                                    r_ps = psa.tile([128, 256], F32,
                                                    name="rps", tag="rps",
                                                    bufs=1)
                                    nc.tensor.matmul(r_ps[:], ones_row[:],
                                                     r_sb[:], start=True,
                                                     stop=True)
                                    nc.scalar.activation(
                                        rbc[:], r_ps[:],
                                        mybir.ActivationFunctionType.Copy)
                                    held.append((dc2, pvps))
                                    for d, p in held:
                                        emit_evac(d, p)
                                    held = []
                                else:
                                    emit_evac(dc2, pvps)

                # ---- o-proj slot0 (bf16) then slot1 (fp8 DR) ----
                # eb-groups of 4 share one [128, 2048] weight tile so each
                # DMA moves 2-4KB per partition row (descriptor-rate bound)
                with (
                    tc.tile_pool(name="pc", bufs=2) as pc,
                    tc.tile_pool(name="psc", bufs=2, space="PSUM") as psc,
                ):
                    for g in range(2):
                        ops = [psc.tile([128, 512], F32, name="ops",
                                        tag="ops", bufs=8)
                               for _ in range(8)]
                        for t in range(KC // 2):
                            wot = pc.tile([128, 2, 2048], BF16, name="wot",
                                          tag="wot", bufs=3)
                            eng = nc.sync if t % 2 == 0 else nc.scalar
                            eng.dma_start(
                                wot[:],
                                wo_d[:, 65536 * g + 4096 * t:
                                     65536 * g + 4096 * (t + 1)].rearrange(
                                    "p (ki c) -> p ki c", ki=2))
                            for ki in range(2):
                                kc = 2 * t + ki
                                for e in range(4):
                                    for ic in range(2):
                                        nc.tensor.matmul(
                                            ops[2 * e + ic][:],
                                            attnT0[kc][:, 128 * ic:
                                                       128 * (ic + 1)],
                                            wot[:, ki, 512 * e:512 * (e + 1)],
                                            start=(kc == 0),
                                            stop=(kc == KC - 1))
                        for ic in range(2):
                            ot = pc.tile([128, 2048], F32, name="ot0",
                                         tag="ot", bufs=2)
                            for e in range(4):
                                nc.vector.tensor_copy(
                                    ot[:, 512 * e:512 * (e + 1)],
                                    ops[2 * e + ic][:])
                            nc.sync.dma_start(
                                out_d[128 * ic:128 * (ic + 1),
                                      2048 * g:2048 * (g + 1)], ot[:])
                    for g in range(2):
                        ops = [psc.tile([128, 512], F32, name="ops",
                                        tag="ops", bufs=8)
                               for _ in range(8)]
                        for t in range(KC // 2):
                            w8g = pc.tile([128, 2, 2048], F8, name="w8g",
                                          tag="w8g", bufs=2)
                            eng = nc.sync if t % 2 == 0 else nc.scalar
                            eng.dma_start(
                                w8g[:],
                                wo8_d[128 * t:128 * (t + 1),
                                      4096 * g:4096 * (g + 1)].rearrange(
                                    "p (i c) -> p i c", i=2))
                            for e in range(4):
                                for ic in range(2):
                                    nc.tensor.matmul(
                                        ops[2 * e + ic][:],
                                        attnT8[t][:, :,
                                                  128 * ic:128 * (ic + 1)],
                                        w8g[:, :, 512 * e:512 * (e + 1)],
                                        start=(t == 0),
                                        stop=(t == KC // 2 - 1),
                                        perf_mode=DR)
                        for ic in range(2):
                            ot = pc.tile([128, 2048], F32, name="ot1",
                                         tag="ot", bufs=2)
                            for e in range(4):
                                nc.vector.tensor_copy(
                                    ot[:, 512 * e:512 * (e + 1)],
                                    ops[2 * e + ic][:])
                            nc.sync.dma_start(
                                out_d[256 + 128 * ic:256 + 128 * (ic + 1),
                                      2048 * g:2048 * (g + 1)], ot[:])

    nc.compile()
    return nc


_BUILD_CACHE = {}

# core r (within its batch group) handles global 256-row i-blocks (r, 7-r)
GMAP = [(r, 7 - r) for r in range(4)]


def _classify_mask(mask):
    """Union-classify each (local block lb, jc) over the 4 quarter cores.

    Returns (cats, per-core packed mask tile arrays, n_mixed). The program
    structure (cats) is shared by all cores; mask tiles are per-core data.
    """
    m = np.asarray(mask).reshape(S, S)  # [i, j]
    cats = {}
    tiles = [[] for _ in range(4)]
    n = 0
    for lb in range(2):
        for jc in range(JC):
            blks = [m[256 * GMAP[r][lb]:256 * (GMAP[r][lb] + 1),
                      128 * jc:128 * (jc + 1)] for r in range(4)]
            if all(np.all(b <= -1e8) for b in blks):
                cats[(lb, jc)] = "skip"
            elif not any(b.any() for b in blks):
                cats[(lb, jc)] = "clean"
            else:
                cats[(lb, jc)] = n
                n += 1
                for r in range(4):
                    # [j, i] orientation, prescaled by 1/SCALE so the ACT's
                    # uniform SCALE reproduces reference's scores*SCALE + mask
                    tiles[r].append(
                        np.ascontiguousarray(blks[r].T) * (1.0 / SCALE))
    maskps = [
        np.concatenate(t, axis=0).astype(np.float32) if t
        else np.zeros((128, 256), np.float32) for t in tiles]
    return cats, maskps, n


def _pack_pdim(x, cols):
    """[K, cols] -> [128, K/128 * cols]: kc-chunk-major per partition, so
    each SBUF tile DMA is one contiguous multi-KB chunk per partition row."""
    k = x.shape[0]
    return np.ascontiguousarray(
        x.reshape(k // 128, 128, cols).transpose(1, 0, 2).reshape(128, -1))


def _pack_wq(w):
    """[4096, 4096] -> [128, 131072]: col kcg*16384 + hp*4096 + ki*1024 + c
    so each (kcg, hp) weight tile is one 8KB chunk per partition row."""
    return np.ascontiguousarray(
        w.reshape(8, 4, 128, 4, 1024).transpose(2, 0, 3, 1, 4).reshape(
            128, -1))


def _pack_wo(w):
    """[4096, 4096] -> [128, 131072]: col g*65536 + kc*2048 + e so each
    (g, kc-pair) weight tile is one 8KB chunk per partition row."""
    return np.ascontiguousarray(
        w.reshape(32, 128, 2, 2048).transpose(1, 2, 0, 3).reshape(128, -1))


def _pack_wo8(w):
    """[4096, 4096] -> [2048, 8192]: row 128t+p, col g*4096 + plane*2048
    + c, so each (t, g) load is one contiguous 4KB chunk per row."""
    return np.ascontiguousarray(
        w.reshape(16, 2, 128, 2, 2048).transpose(0, 2, 3, 1, 4).reshape(
            2048, 8192))


def _pack_hs8(x):
    """[4096, 256] -> [128, 8192]: row p, col t*512 + plane*256 + c."""
    return np.ascontiguousarray(
        x.reshape(16, 2, 128, 256).transpose(2, 0, 1, 3).reshape(128, 8192))


def kernel(hidden_states, attention_mask, Wq, Wk, Wv, Wo, trace=False):
    global LAST_RESULTS
    bf = ml_dtypes.bfloat16

    cats, maskps, n_mixed = _classify_mask(attention_mask)
    key = tuple(sorted((k, v if isinstance(v, str) else "m")
                       for k, v in cats.items()))
    if key not in _BUILD_CACHE:
        _BUILD_CACHE[key] = _build(cats, n_mixed)
    nc = _BUILD_CACHE[key]

    # deinterleave rope pairs within each head's 1024 columns
    perm = np.concatenate([np.arange(0, D, 2), np.arange(1, D, 2)])
    cols = np.concatenate([h * D + perm for h in range(NH)])
    wq_p = np.ascontiguousarray(Wq[:, cols])
    wq_bf = _pack_wq(wq_p.astype(bf))
    wk_p = _pack_pdim(np.ascontiguousarray(Wk[:, perm]).astype(bf), D)
    wv_c = _pack_pdim(np.asarray(Wv).astype(bf), D)
    wo_c = _pack_wo(np.asarray(Wo).astype(bf))
    wo8 = _pack_wo8(np.asarray(Wo).astype(NP_F8))

    freqs = 1.0 / (10000.0 ** (np.arange(0, D, 2, dtype=np.float64) / D))
    ang = np.outer(np.arange(S, dtype=np.float64), freqs)  # [S, PD]
    cosT = np.ascontiguousarray(np.cos(ang).T).astype(bf)  # [PD, S]
    sinT = np.ascontiguousarray(np.sin(ang).T).astype(bf)

    hsT = [np.ascontiguousarray(hidden_states[b].T) for b in range(B)]
    hsT_bf = [h.astype(bf) for h in hsT]

    in_maps = []
    for c in range(NCORES):
        b, r = c // 4, c % 4
        g0, g1 = GMAP[r]
        icols = np.r_[256 * g0:256 * (g0 + 1), 256 * g1:256 * (g1 + 1)]
        hs8 = _pack_hs8(
            np.ascontiguousarray(
                hsT[b][:, 256 * g1:256 * (g1 + 1)]).astype(NP_F8))
        in_maps.append({
            "hsq": _pack_pdim(
                np.ascontiguousarray(hsT_bf[b][:, Q * r:Q * (r + 1)]), Q),
            "hsq2": _pack_pdim(
                np.ascontiguousarray(
                    hsT_bf[b][:, 256 * g0:256 * (g0 + 1)]), 256),
            "hs8": hs8,
            "wq": wq_bf,
            "wk": wk_p,
            "wv": wv_c,
            "wo": wo_c,
            "wo8": wo8,
            "cosq": np.ascontiguousarray(cosT[:, Q * r:Q * (r + 1)]),
            "sinq": np.ascontiguousarray(sinT[:, Q * r:Q * (r + 1)]),
            "cosq2": np.ascontiguousarray(cosT[:, icols]),
            "sinq2": np.ascontiguousarray(sinT[:, icols]),
            "maskp": maskps[r],
        })

    res = bass_utils.run_bass_kernel_spmd(
        nc, in_maps, core_ids=list(range(NCORES)), trace=trace)
    LAST_RESULTS = res

    out = np.empty((B, S, H), np.float32)
    for c in range(NCORES):
        b, r = c // 4, c % 4
        g0, g1 = GMAP[r]
        o = res.results[c]["out"]
        out[b, 256 * g0:256 * (g0 + 1), :] = o[0:256]
        out[b, 256 * g1:256 * (g1 + 1), :] = o[256:512]
    return out
